# revision 1
# baseline (speedup 1.0000x reference)
"""DiffHead (differential attention head) Trainium2 Bass kernel.

Strategy (hardcoded for B=8, T=2048, C=1024, HS=128, 8 cores):
  - Data-parallel over batch: one batch element per NeuronCore.
  - Host side only reshapes/shards: per-core q/k/v slices are passed
    transposed ([C, T]) so the projection matmuls can contract over C on
    the partition axis. All FLOPs run on device.
  - Scores are computed transposed (S^T[k, q]) so softmax normalization
    uses matmul column-sums; masked fills of 1e-9 scale to exactly 1.0f
    after exp (exp(1e-9*scale) == 1.0f), so the fully-masked block
    region (k > q+1, beyond the diagonal/superdiagonal blocks) is never
    computed: its contributions are closed-form (suffix sums of V rows
    and a masked-count per query tile).
  - Softmax normalization is deferred to after the U@V products (per
    output element, not per score element). lambda is computed on
    device from lq/lk vectors.
"""

import numpy as np
import ml_dtypes

try:
    import concourse.bacc as bacc
except ImportError:  # pragma: no cover
    import sys

    sys.path.insert(0, "/opt/trn_rl_repo")
    import concourse.bacc as bacc

import concourse.mybir as mybir
import concourse.tile as tile
from concourse.bass_utils import run_bass_kernel_spmd

F32 = mybir.dt.float32
F32R = mybir.dt.float32r
BF16 = mybir.dt.bfloat16
EXP = mybir.ActivationFunctionType.Exp

HS = 128
LAMBDA_INIT = 0.8
N_CORES = 8
INPUT_BF16 = True


def _r(ap):
    """View an f32 AP as float32r so the PE runs at full rate."""
    return ap.bitcast(F32R)


def build_nc(T=2048, C=1024, NQ=512, repeat=1, phase1_only=False):
    """Build the per-core Bass program. Same NEFF on all 8 cores (SPMD).

    repeat > 1 wraps the body in a hardware loop (for wall-clock slope
    timing); results are identical since the body is idempotent.
    """
    import contextlib

    nT = T // 128
    nC = C // 128
    NQ = min(NQ, T)
    SCALE = float(HS) ** -0.5

    nc = bacc.Bacc("TRN2", target_bir_lowering=False, num_devices=N_CORES)

    XDT = BF16 if INPUT_BF16 else F32
    qT = nc.dram_tensor("qT", [C, T], XDT, kind="ExternalInput")
    kT = nc.dram_tensor("kT", [C, T], XDT, kind="ExternalInput")
    vT = nc.dram_tensor("vT", [C, T], XDT, kind="ExternalInput")
    wq = nc.dram_tensor("wq", [C, 2 * HS], XDT, kind="ExternalInput")
    wk = nc.dram_tensor("wk", [C, 2 * HS], XDT, kind="ExternalInput")
    wv = nc.dram_tensor("wv", [C, HS], XDT, kind="ExternalInput")
    pad = nc.dram_tensor("pad", [1, T], F32, kind="ExternalInput")
    npad = nc.dram_tensor("npad", [1, T], F32, kind="ExternalInput")
    lvec = nc.dram_tensor("lvec", [HS, 4], F32, kind="ExternalInput")
    trid = nc.dram_tensor("trid", [128, 128], F32, kind="ExternalInput")
    tris = nc.dram_tensor("tris", [128, 128], F32, kind="ExternalInput")
    # [tris | trid] side by side: masks adjacent superdiag+diag tiles in one op
    trisd = nc.dram_tensor("trisd", [128, 256], F32, kind="ExternalInput")
    idb = nc.dram_tensor("idb", [128, 128], BF16, kind="ExternalInput")
    idf = nc.dram_tensor("idf", [128, 128], F32, kind="ExternalInput")
    onesb = nc.dram_tensor("onesb", [128, 1], BF16, kind="ExternalInput")
    onesr = nc.dram_tensor("onesr", [1, 128], F32, kind="ExternalInput")
    sufcnt = nc.dram_tensor("sufcnt", [1, T], F32, kind="ExternalInput")
    # output stays transposed ([dv, T]); the host un-transposes when
    # stacking per-core results (layout marshaling, same as the inputs).
    out = nc.dram_tensor("out", [HS, T], F32, kind="ExternalOutput")

    with tile.TileContext(nc) as tc:
        rep_cm = tc.For_i(0, repeat, 1) if repeat > 1 else contextlib.nullcontext()
        with (
            rep_cm,
            tc.tile_pool(name="consts", bufs=1) as consts,
            tc.tile_pool(name="persist", bufs=1) as persist,
        ):
            # ---- constants ----
            trid_sb = consts.tile([128, 128], F32, tag="trid")
            nc.sync.dma_start(trid_sb, trid.ap())
            tris_sb = consts.tile([128, 128], F32, tag="tris")
            nc.sync.dma_start(tris_sb, tris.ap())
            trisd_sb = consts.tile([128, 256], F32, tag="trisd")
            nc.sync.dma_start(trisd_sb, trisd.ap())
            idb_sb = consts.tile([128, 128], BF16, tag="idb")
            nc.sync.dma_start(idb_sb, idb.ap())
            idf_sb = consts.tile([128, 128], F32, tag="idf")
            nc.sync.dma_start(idf_sb, idf.ap())
            onesb_sb = consts.tile([128, 1], BF16, tag="onesb")
            nc.sync.dma_start(onesb_sb, onesb.ap())
            onesr_sb = consts.tile([1, 128], F32, tag="onesr")
            nc.sync.dma_start(onesr_sb, onesr.ap())
            sufcnt_sb = consts.tile([1, T], F32, tag="sufcnt")
            nc.sync.dma_start(sufcnt_sb, sufcnt.ap())
            pad_sb = consts.tile([1, T], F32, tag="pad")
            nc.sync.dma_start(pad_sb, pad.ap())
            npad_sb = consts.tile([1, T], F32, tag="npad")
            nc.sync.dma_start(npad_sb, npad.ap())
            lv_sb = consts.tile([HS, 4], F32, tag="lv")
            nc.sync.dma_start(lv_sb, lvec.ap())
            # weights, blocked by contraction tile: [128, nC, d]
            WDT = BF16 if INPUT_BF16 else F32R
            _w = (lambda ap: ap) if INPUT_BF16 else _r
            wq_sb = consts.tile([128, nC, 2 * HS], WDT, tag="wq")
            nc.sync.dma_start(wq_sb, _w(wq.ap().rearrange("(ct p) d -> p ct d", p=128)))
            wk_sb = consts.tile([128, nC, 2 * HS], WDT, tag="wk")
            nc.sync.dma_start(wk_sb, _w(wk.ap().rearrange("(ct p) d -> p ct d", p=128)))
            wv_sb = consts.tile([128, nC, HS], WDT, tag="wv")
            nc.sync.dma_start(wv_sb, _w(wv.ap().rearrange("(ct p) d -> p ct d", p=128)))

            # ---- persistent intermediates ----
            q1t = persist.tile([128, T], F32R, tag="q1t")  # Q1^T [d, t]
            q2t = persist.tile([128, T], F32R, tag="q2t")
            k1t = persist.tile([128, T], F32R, tag="k1t")
            k2t = persist.tile([128, T], F32R, tag="k2t")
            vsb = persist.tile([128, nT, 128], BF16, tag="vsb")  # V natural, blocked
            vtb = persist.tile([128, T], BF16, tag="vtb")  # V^T bf16
            padbc = persist.tile([128, T], F32, tag="padbc")
            sufv = persist.tile([128, nT], F32, tag="sufv")
            tvn = persist.tile([128, 1], F32, tag="tvn")  # totalV / T
            lamc = persist.tile([128, 1], F32, tag="lamc")  # lambda, bcast col

            # ================= phase 1: projections =================
            with (
                tc.tile_pool(name="xs", bufs=3) as xs,
                tc.tile_pool(name="pp1", bufs=2, space="PSUM") as pp1,
            ):
                # V first: phase 1.5 (V transpose/sums) overlaps the k/q
                # streams, and phase 2 needs V from its first AV matmul.
                vps = pp1.tile([128, T], F32, tag="proj")
                for ct in range(nC):
                    xt = xs.tile([128, T], WDT, tag="xt")
                    for n0 in range(0, T, 512):
                        nc.sync.dma_start(
                            xt[:, n0 : n0 + 512],
                            _w(vT.ap()[ct * 128 : (ct + 1) * 128, n0 : n0 + 512]),
                        )
                    for n0 in range(0, T, 512):
                        nc.tensor.matmul(
                            vps[:, n0 : n0 + 512],
                            wv_sb[:, ct, :],
                            xt[:, n0 : n0 + 512],
                            start=(ct == 0),
                            stop=(ct == nC - 1),
                        )
                for n0 in range(0, T, 512):
                    nc.scalar.copy(vtb[:, n0 : n0 + 512], vps[:, n0 : n0 + 512])
                for xdram, w_sb, outs in (
                    (kT, wk_sb, (k1t, k2t)),
                    (qT, wq_sb, (q1t, q2t)),
                ):
                    ps = [
                        pp1.tile([128, T], F32, tag="proj", name=f"ps{h}")
                        for h in range(2)
                    ]
                    for ct in range(nC):
                        xt = xs.tile([128, T], WDT, tag="xt")
                        for n0 in range(0, T, 512):
                            nc.sync.dma_start(
                                xt[:, n0 : n0 + 512],
                                _w(
                                    xdram.ap()[
                                        ct * 128 : (ct + 1) * 128, n0 : n0 + 512
                                    ]
                                ),
                            )
                        for h in range(2):
                            for n0 in range(0, T, 512):
                                nc.tensor.matmul(
                                    ps[h][:, n0 : n0 + 512],
                                    w_sb[:, ct, h * HS : (h + 1) * HS],
                                    xt[:, n0 : n0 + 512],
                                    start=(ct == 0),
                                    stop=(ct == nC - 1),
                                )
                    for h in range(2):
                        for ni, n0 in enumerate(range(0, T, 512)):
                            if (h + ni) % 2 == 0:
                                nc.scalar.copy(
                                    outs[h][:, n0 : n0 + 512],
                                    ps[h][:, n0 : n0 + 512],
                                )
                            else:
                                nc.vector.tensor_copy(
                                    outs[h][:, n0 : n0 + 512],
                                    ps[h][:, n0 : n0 + 512],
                                )

            # ============ phase 1.5: V natural, sums, lambda, pad bcast ============
            with tc.tile_pool(name="ppt", bufs=2, space="PSUM") as ppt:
                # V^T -> V natural blocks (PE transpose, bf16)
                for j in range(nT):
                    vtr = ppt.tile([128, 128], BF16, tag="m")
                    nc.tensor.transpose(vtr, vtb[:, j * 128 : (j + 1) * 128], idb_sb)
                    nc.vector.tensor_copy(vsb[:, j, :], vtr)
                # per-block column sums of V rows: vcol[:, j] = sum_k V[k, :]
                vcol_ps = ppt.tile([128, nT], F32, tag="vc")
                for j in range(nT):
                    nc.tensor.matmul(
                        vcol_ps[:, j : j + 1],
                        vsb[:, j, :],
                        onesb_sb,
                        start=True,
                        stop=True,
                    )
                vcols = consts.tile([128, nT], F32, tag="vcols")
                nc.vector.tensor_copy(vcols, vcol_ps)
                # suffix sums: sufv[:, i] = sum_{j >= i+2} vcol[:, j]
                nc.vector.memset(sufv[:, nT - 1 : nT], 0.0)
                if nT >= 2:
                    nc.vector.memset(sufv[:, nT - 2 : nT - 1], 0.0)
                for i in range(nT - 3, -1, -1):
                    nc.vector.tensor_add(
                        sufv[:, i : i + 1], sufv[:, i + 1 : i + 2], vcols[:, i + 2 : i + 3]
                    )
                nc.vector.tensor_add(tvn, sufv[:, 0:1], vcols[:, 0:1])
                if nT >= 2:
                    nc.vector.tensor_add(tvn, tvn, vcols[:, 1:2])
                nc.scalar.mul(tvn, tvn, 1.0 / T)

                # lambda = exp(lq1.lk1) - exp(lq2.lk2) + LAMBDA_INIT
                dots_ps = ppt.tile([1, 2], F32, tag="vc")
                nc.tensor.matmul(
                    dots_ps[:, 0:1], lv_sb[:, 0:1], lv_sb[:, 1:2],
                    start=True, stop=True,
                )
                nc.tensor.matmul(
                    dots_ps[:, 1:2], lv_sb[:, 2:3], lv_sb[:, 3:4],
                    start=True, stop=True,
                )
                eexp = consts.tile([1, 2], F32, tag="eexp")
                nc.scalar.activation(eexp, dots_ps, EXP)
                lam1 = consts.tile([1, 1], F32, tag="lam1")
                nc.vector.tensor_sub(lam1, eexp[:, 0:1], eexp[:, 1:2])
                nc.vector.tensor_scalar_add(lam1, lam1, LAMBDA_INIT)
                lbc_ps = ppt.tile([128, 1], F32, tag="vc")
                nc.tensor.matmul(lbc_ps, onesr_sb, lam1, start=True, stop=True)
                nc.vector.tensor_copy(lamc, lbc_ps)
                # pad rows see uniform attention in both branches:
                # out = (1 - lambda) * mean(V)
                tvl = consts.tile([128, 1], F32, tag="tvl")
                nc.vector.tensor_scalar(
                    tvl, tvn, lamc, None, mybir.AluOpType.mult
                )
                nc.vector.tensor_sub(tvn, tvn, tvl)

                # pad row masks broadcast across partitions
                for n0 in range(0, T, 512):
                    w_ = min(512, T - n0)
                    pbc = ppt.tile([128, NQ], F32, tag="m")
                    nc.tensor.matmul(
                        pbc[:, :w_], onesr_sb, pad_sb[:, n0 : n0 + w_],
                        start=True, stop=True,
                    )
                    nc.scalar.copy(padbc[:, n0 : n0 + w_], pbc[:, :w_])

            # ================= phase 2: attention =================
            with (

                tc.tile_pool(name="psS", bufs=2, space="PSUM") as psS,
                tc.tile_pool(name="psO", bufs=2, space="PSUM") as psO,
                tc.tile_pool(name="psD", bufs=1, space="PSUM") as psD,
                tc.tile_pool(name="psM", bufs=1, space="PSUM") as psM,
                tc.tile_pool(name="sb2", bufs=3) as sb2,
                tc.tile_pool(name="sb2b", bufs=2) as sb2b,
                tc.tile_pool(name="obufs", bufs=4) as obufs,
            ):
                pending = []

                def emit_combine(q0, br_res):
                    # combine branches: oT = O1*R1 - lambda*O2*R2, fix pad
                    # rows to uniform attention, transpose to natural, store.
                    (ou1sb, r1bc), (ou2sb, r2bc) = br_res
                    c1 = sb2b.tile([128, NQ], F32, tag="c1")
                    nc.vector.tensor_mul(c1, ou1sb, r1bc)
                    c2 = sb2b.tile([128, NQ], F32, tag="c2")
                    nc.vector.scalar_tensor_tensor(
                        c2, ou2sb, lamc, r2bc,
                        mybir.AluOpType.mult, mybir.AluOpType.mult,
                    )
                    oT = sb2b.tile([128, NQ], F32, tag="oT")
                    nc.vector.tensor_sub(oT, c1, c2)
                    # padded columns: oT is 0 there (rrow was masked); add the
                    # uniform value (1-lambda)*mean(V) in one fused op
                    oT2 = sb2b.tile([128, NQ], F32, tag="oT2")
                    nc.vector.scalar_tensor_tensor(
                        oT2, padbc[:, q0 : q0 + NQ], tvn, oT,
                        mybir.AluOpType.mult, mybir.AluOpType.add,
                    )
                    nc.sync.dma_start(out.ap()[:, q0 : q0 + NQ], oT2)

                if phase1_only:
                    nc.sync.dma_start(out.ap()[:, :], q1t.bitcast(F32))
                for c in range(0 if phase1_only else T // NQ):
                    q0 = c * NQ
                    i_hi = (q0 + NQ) // 128  # exclusive tile bound of chunk
                    jmax = min(i_hi, nT - 1)  # inclusive max j
                    # both branches' denominator rows share one PSUM bank
                    # (partitions 0 and 32)
                    d2ps = psD.tile([128, NQ], F32, tag="dd")
                    # k-tiles that cover the full chunk width get paired so
                    # one wide exp serves two score blocks (ACT has a ~300ns
                    # fixed cost per op)
                    full = [j for j in range(jmax + 1) if 128 * (j - 1) <= q0]
                    part = [j for j in range(jmax + 1) if 128 * (j - 1) > q0]
                    groups = [full[i : i + 2] for i in range(0, len(full), 2)]
                    groups += [[j] for j in part]
                    ou_h = []
                    for br in range(2):
                        QT = q1t if br == 0 else q2t
                        KT = k1t if br == 0 else k2t
                        ou = psO.tile([128, NQ], F32, tag="ou")
                        ou_h.append(ou)
                        dps = d2ps[32 * br : 32 * br + 1, :]
                        for grp in groups:
                            s_ps = psS.tile([128, 2 * NQ], F32, tag="s")
                            u_t = sb2.tile([128, 2 * NQ], BF16, tag="u")
                            spans = []
                            for gi, j in enumerate(grp):
                                q_lo = max(q0, 128 * (j - 1))
                                off = q_lo - q0
                                w_ = q0 + NQ - q_lo
                                base = gi * NQ
                                spans.append((j, base, off, w_))
                                nc.tensor.matmul(
                                    s_ps[:, base + off : base + off + w_],
                                    KT[:, j * 128 : (j + 1) * 128],
                                    QT[:, q_lo : q_lo + w_],
                                    start=True,
                                    stop=True,
                                )
                                # triangular masks on superdiag/diag tiles
                                # (-> score 0 -> U=1)
                                i0t, i1t = q_lo // 128, i_hi
                                has_s = i0t <= j - 1 < i1t
                                has_d = i0t <= j < i1t
                                if has_s and has_d:
                                    coff = base + (j - 1) * 128 - q0
                                    nc.vector.tensor_mul(
                                        s_ps[:, coff : coff + 256],
                                        s_ps[:, coff : coff + 256],
                                        trisd_sb,
                                    )
                                elif has_s:
                                    coff = base + (j - 1) * 128 - q0
                                    nc.vector.tensor_mul(
                                        s_ps[:, coff : coff + 128],
                                        s_ps[:, coff : coff + 128],
                                        tris_sb,
                                    )
                                elif has_d:
                                    coff = base + j * 128 - q0
                                    nc.vector.tensor_mul(
                                        s_ps[:, coff : coff + 128],
                                        s_ps[:, coff : coff + 128],
                                        trid_sb,
                                    )
                            lo = spans[0][1] + spans[0][2]
                            hi = spans[-1][1] + spans[-1][2] + spans[-1][3]
                            nc.scalar.activation(
                                u_t[:, lo:hi], s_ps[:, lo:hi], EXP, scale=SCALE
                            )
                            # U@V and column-sum accumulation. j=0 covers the
                            # whole chunk, so start=True there zeroes the full
                            # bank; later j accumulate into sub-ranges. The
                            # sim's coarse per-bank group check can't follow
                            # that, hence skip_group_check.
                            for j, base, off, w_ in spans:
                                nc.tensor.matmul(
                                    ou[:, off : off + w_],
                                    vsb[:, j, :],
                                    u_t[:, base + off : base + off + w_],
                                    start=(j == 0),
                                    stop=(j == jmax),
                                    skip_group_check=True,
                                )
                                nc.tensor.matmul(
                                    dps[:, off : off + w_],
                                    onesb_sb,
                                    u_t[:, base + off : base + off + w_],
                                    start=(j == 0),
                                    stop=(j == jmax),
                                    skip_group_check=True,
                                )
                    # per-branch boundary: denominators -> reciprocal ->
                    # broadcast, and O_u drain (+suffix V sums). Emitted after
                    # BOTH branches' j-loops so the PE's in-order stream never
                    # stalls waiting on the DVE reciprocal chain.
                    br_res = []
                    for br in range(2):
                        ou = ou_h[br]
                        dps = d2ps[32 * br : 32 * br + 1, :]
                        dsb = sb2.tile([1, NQ], F32, tag="drow")
                        nc.vector.tensor_add(dsb, dps, sufcnt_sb[:, q0 : q0 + NQ])
                        rrow = sb2.tile([1, NQ], F32, tag="rrow")
                        nc.vector.reciprocal(rrow, dsb)
                        # zero padded columns so the combine needs no
                        # separate (1 - pad) pass
                        nc.vector.tensor_mul(rrow, rrow, npad_sb[:, q0 : q0 + NQ])
                        rbc_ps = psM.tile([128, NQ], F32, tag="m")
                        nc.tensor.matmul(
                            rbc_ps, onesr_sb, rrow, start=True, stop=True
                        )
                        rbc = obufs.tile([128, NQ], F32, tag="rbc")
                        nc.vector.tensor_copy(rbc, rbc_ps)
                        # drain O_u adding the skipped-region suffix V sums
                        ousb = obufs.tile([128, NQ], F32, tag="ousb")
                        for i in range(q0 // 128, i_hi):
                            coff = i * 128 - q0
                            nc.vector.tensor_scalar_add(
                                ousb[:, coff : coff + 128],
                                ou[:, coff : coff + 128],
                                sufv[:, i : i + 1],
                            )
                        br_res.append((ousb, rbc))
                    # defer the combine by one chunk: its DVE chain and PE
                    # transposes overlap the NEXT chunk's j-loops.
                    pending.append((q0, br_res))
                    todo = []
                    if len(pending) > 1:
                        todo = [pending.pop(0)]
                    if c == T // NQ - 1:
                        todo += pending
                        pending = []
                    for q0c, br_resc in todo:
                        emit_combine(q0c, br_resc)


    nc.compile()
    return nc


def _host_constants(T, nT):
    kl = np.arange(128)[:, None]
    ql = np.arange(128)[None, :]
    trid = (kl <= ql + 1).astype(np.float32)
    tris = ((kl + 128) <= (ql + 1)).astype(np.float32)
    idf = np.eye(128, dtype=np.float32)
    idb = np.eye(128, dtype=ml_dtypes.bfloat16)
    onesb = np.ones((128, 1), dtype=ml_dtypes.bfloat16)
    onesr = np.ones((1, 128), dtype=np.float32)
    tiles = np.arange(T) // 128
    sufcnt = (128.0 * np.maximum(0, nT - 2 - tiles)).astype(np.float32)[None, :]
    return dict(
        trid=trid, tris=tris, trisd=np.concatenate([tris, trid], axis=1),
        idf=idf, idb=idb, onesb=onesb, onesr=onesr, sufcnt=sufcnt,
    )


_NC_CACHE = {}


def make_in_maps(q, k, v, pad_mask, Wq, Wk, Wv, lq1, lk1, lq2, lk2):
    """Per-core input dicts (host-side sharding + layout marshaling)."""
    B, T, C = q.shape
    xdt = ml_dtypes.bfloat16 if INPUT_BF16 else np.float32
    consts = _host_constants(T, T // 128)
    lvec = np.stack(
        [np.asarray(lq1), np.asarray(lk1), np.asarray(lq2), np.asarray(lk2)], axis=1
    ).astype(np.float32)
    wq_ = np.asarray(Wq).astype(xdt)
    wk_ = np.asarray(Wk).astype(xdt)
    wv_ = np.asarray(Wv).astype(xdt)

    in_maps = []
    for b in range(B):
        padf = np.asarray(pad_mask[b], dtype=np.float32)[None, :]
        in_maps.append(
            dict(
                qT=np.ascontiguousarray(np.asarray(q[b]).T.astype(xdt)),
                kT=np.ascontiguousarray(np.asarray(k[b]).T.astype(xdt)),
                vT=np.ascontiguousarray(np.asarray(v[b]).T.astype(xdt)),
                wq=wq_, wk=wk_, wv=wv_,
                pad=padf, npad=1.0 - padf,
                lvec=lvec,
                **consts,
            )
        )
    return in_maps


def kernel(q, k, v, pad_mask, Wq, Wk, Wv, lq1, lk1, lq2, lk2):
    B, T, C = q.shape
    assert B == N_CORES
    key = (T, C)
    if key not in _NC_CACHE:
        _NC_CACHE[key] = build_nc(T=T, C=C)
    nc = _NC_CACHE[key]
    in_maps = make_in_maps(q, k, v, pad_mask, Wq, Wk, Wv, lq1, lk1, lq2, lk2)
    res = run_bass_kernel_spmd(nc, in_maps, core_ids=list(range(N_CORES)))
    return np.stack(
        [np.ascontiguousarray(r["out"].T) for r in res.results], axis=0
    )



# revision 14
# speedup vs baseline: 1.1789x; 1.1789x over previous
"""DiffHead (differential attention head) Trainium2 Bass kernel.

Strategy (hardcoded for B=8, T=2048, C=1024, HS=128, 8 cores):
  - Data-parallel over batch: one batch element per NeuronCore.
  - Host side only reshapes/shards: per-core q/k/v slices are passed
    transposed ([C, T]) so the projection matmuls contract over C on the
    partition axis. All FLOPs run on device.
  - Scores are computed transposed (S^T[k, q]); masked fills of 1e-9
    scale to exactly 1.0f after exp, so the fully-masked region beyond
    the diagonal/superdiagonal blocks is never computed: its
    contributions are closed-form (suffix sums of V rows + a masked
    count folded into the denominator matmul chain).
  - Phase 2 is split into stage A (scores -> mask -> exp into an SBUF
    "u band" per branch) and stage B (U@V + denominator matmuls +
    combine), manually interleaved so the PE never waits long on the
    ACT exp stream. The V projection itself interleaves with the first
    stage-A group to hide its DMA pacing.
  - All small constants + weights ship in packed blobs and inputs load
    one DMA per 128-row tile, keeping HWDGE descriptor generation off
    the critical path.
  - Row->all-partition broadcasts (reciprocal rows, pad row, lambda)
    and part of the combine run on the otherwise idle GpSimd engine.
"""

import numpy as np
import ml_dtypes

try:
    import concourse.bacc as bacc
except ImportError:  # pragma: no cover
    import sys

    sys.path.insert(0, "/opt/trn_rl_repo")
    import concourse.bacc as bacc

import concourse.mybir as mybir
import concourse.tile as tile
from concourse.bass_utils import run_bass_kernel_spmd

F32 = mybir.dt.float32
F32R = mybir.dt.float32r
BF16 = mybir.dt.bfloat16
EXP = mybir.ActivationFunctionType.Exp

HS = 128
LAMBDA_INIT = 0.8
N_CORES = 8

# feature flags (fallbacks if an op turns out unsupported)
MASK_ON_POOL = False
COMBINE_ON_POOL = True


def _r(ap):
    """View an f32 AP as float32r so the PE runs at full rate."""
    return ap.bitcast(F32R)


def _band_widths(T, nT):
    """Score-band column width per k-tile j: queries q >= 128*(j-1)."""
    return [T - (0 if j == 0 else 128 * (j - 1)) for j in range(nT)]


def build_nc(T=2048, C=1024, NQ=512, repeat=1, phase1_only=False):
    """Build the per-core Bass program. Same NEFF on all 8 cores (SPMD).

    repeat > 1 wraps the body in a hardware loop (for wall-clock slope
    timing); results are identical since the body is idempotent.
    """
    import contextlib

    nT = T // 128
    nC = C // 128
    NQ = min(NQ, T)
    SCALE = float(HS) ** -0.5

    widths = _band_widths(T, nT)
    band_off = [0] * nT
    for j in range(1, nT):
        band_off[j] = band_off[j - 1] + widths[j - 1]
    band_cols = band_off[-1] + widths[-1]

    # K weights ship alone (needed first); the rest packs into cbR:
    # [wq | wv | idb | onesb]
    KCOLS = nC * 2 * HS
    WQ0 = 0
    WV0 = WQ0 + nC * 2 * HS
    IDB0 = WV0 + nC * HS
    ONE0 = IDB0 + 128
    RCOLS = ONE0 + 1
    # packed f32 blob offsets: [tris | trid | lvec]
    TRIS0, TRID0, LV0 = 0, 128, 256
    F32COLS = LV0 + 4

    nc = bacc.Bacc("TRN2", target_bir_lowering=False, num_devices=N_CORES)

    qT = nc.dram_tensor("qT", [C, T], BF16, kind="ExternalInput")
    kT = nc.dram_tensor("kT", [C, T], BF16, kind="ExternalInput")
    vT = nc.dram_tensor("vT", [C, T], BF16, kind="ExternalInput")
    cbK = nc.dram_tensor("cbK", [128, KCOLS], BF16, kind="ExternalInput")
    cbR = nc.dram_tensor("cbR", [128, RCOLS], BF16, kind="ExternalInput")
    cbf32 = nc.dram_tensor("cbf32", [128, F32COLS], F32, kind="ExternalInput")
    pn2 = nc.dram_tensor("pn2", [1, 2 * T], F32, kind="ExternalInput")
    sufcnt_bf = nc.dram_tensor("sufcnt_bf", [1, T], BF16, kind="ExternalInput")
    # output stays transposed ([dv, T]); the host un-transposes.
    out = nc.dram_tensor("out", [HS, T], F32, kind="ExternalOutput")

    with tile.TileContext(nc) as tc:
        rep_cm = tc.For_i(0, repeat, 1) if repeat > 1 else contextlib.nullcontext()
        with (
            rep_cm,
            tc.tile_pool(name="consts", bufs=1) as consts,
            tc.tile_pool(name="persist", bufs=1) as persist,
        ):
            # ---- packed constants (K weights first: needed immediately) ----
            cbk_sb = consts.tile([128, KCOLS], BF16, tag="cbK")
            nc.sync.dma_start(cbk_sb, cbK.ap())
            cbr_sb = consts.tile([128, RCOLS], BF16, tag="cbR")
            cf = consts.tile([128, F32COLS], F32, tag="cbf32")
            pn_sb = consts.tile([1, 2 * T], F32, tag="pn2")
            sufb_sb = consts.tile([1, T], BF16, tag="sufb")

            def wk_w(ct, h):
                return cbk_sb[:, ct * 2 * HS + h * HS : ct * 2 * HS + (h + 1) * HS]

            def wq_w(ct, h):
                return cbr_sb[:, WQ0 + ct * 2 * HS + h * HS : WQ0 + ct * 2 * HS + (h + 1) * HS]

            def wv_w(ct):
                return cbr_sb[:, WV0 + ct * HS : WV0 + (ct + 1) * HS]

            idb_w = cbr_sb[:, IDB0 : IDB0 + 128]
            onesb_w = cbr_sb[:, ONE0 : ONE0 + 1]
            trisd_sb = cf[:, TRIS0 : TRIS0 + 256]
            trid_sb = cf[:, TRID0 : TRID0 + 128]
            lv_sb = cf[:, LV0 : LV0 + 4]
            pad_sb = pn_sb[:, 0:T]
            npad_sb = pn_sb[:, T : 2 * T]

            # ---- persistent intermediates ----
            q1t = persist.tile([128, T], F32R, tag="q1t")
            q2t = persist.tile([128, T], F32R, tag="q2t")
            k1t = persist.tile([128, T], F32R, tag="k1t")
            k2t = persist.tile([128, T], F32R, tag="k2t")
            vsb = persist.tile([128, nT, 128], BF16, tag="vsb")  # V natural
            padbc = persist.tile([128, T], F32, tag="padbc")
            vcols = persist.tile([128, nT], F32, tag="vcols")
            sufv = persist.tile([128, nT], F32, tag="sufv")
            tvn = persist.tile([128, 1], F32, tag="tvn")  # (1-lambda)*mean(V)
            lamc = persist.tile([128, 1], F32, tag="lamc")
            u_band = [
                persist.tile([128, band_cols], BF16, tag="u1", name="u1"),
                persist.tile([128, band_cols], BF16, tag="u2", name="u2"),
            ]

            mask_eng = nc.gpsimd if MASK_ON_POOL else nc.vector
            comb_eng = nc.gpsimd if COMBINE_ON_POOL else nc.vector

            with tc.tile_pool(name="xs", bufs=4) as xs:
                # ============ K and Q projections (full-PSUM scope) ============
                with tc.tile_pool(name="ppK", bufs=2, space="PSUM") as ppK:
                    first = True
                    for xdram, w_of, outs in (
                        (kT, wk_w, (k1t, k2t)),
                        (qT, wq_w, (q1t, q2t)),
                    ):
                        ps = [
                            ppK.tile([128, T], F32, tag="proj", name=f"ps{h}")
                            for h in range(2)
                        ]
                        for ct in range(nC):
                            xt = xs.tile([128, T], BF16, tag="xt")
                            nc.sync.dma_start(
                                xt, xdram.ap()[ct * 128 : (ct + 1) * 128, :]
                            )
                            if first:
                                # remaining consts ride behind the first tile
                                nc.sync.dma_start(cbr_sb, cbR.ap())
                                nc.sync.dma_start(cf, cbf32.ap())
                                nc.sync.dma_start(pn_sb, pn2.ap())
                                nc.sync.dma_start(sufb_sb, sufcnt_bf.ap())
                                first = False
                            for h in range(2):
                                for n0 in range(0, T, 512):
                                    nc.tensor.matmul(
                                        ps[h][:, n0 : n0 + 512],
                                        w_of(ct, h),
                                        xt[:, n0 : n0 + 512],
                                        start=(ct == 0),
                                        stop=(ct == nC - 1),
                                    )
                        for h in range(2):
                            for ni, n0 in enumerate(range(0, T, 1024)):
                                if (h + ni) % 2 == 0:
                                    nc.scalar.copy(
                                        outs[h][:, n0 : n0 + 1024],
                                        ps[h][:, n0 : n0 + 1024],
                                    )
                                else:
                                    nc.vector.tensor_copy(
                                        outs[h][:, n0 : n0 + 1024],
                                        ps[h][:, n0 : n0 + 1024],
                                    )

                # ============ lambda + pad broadcast (tiny PSUM scope) ============
                with tc.tile_pool(name="ppL", bufs=1, space="PSUM") as ppL:
                    dots_ps = ppL.tile([1, 2], F32, tag="t")
                    nc.tensor.matmul(
                        dots_ps[:, 0:1], lv_sb[:, 0:1], lv_sb[:, 1:2],
                        start=True, stop=True,
                    )
                    nc.tensor.matmul(
                        dots_ps[:, 1:2], lv_sb[:, 2:3], lv_sb[:, 3:4],
                        start=True, stop=True,
                    )
                    eexp = consts.tile([1, 2], F32, tag="eexp")
                    nc.scalar.activation(eexp, dots_ps, EXP)
                    lam1 = consts.tile([1, 1], F32, tag="lam1")
                    nc.vector.tensor_sub(lam1, eexp[:, 0:1], eexp[:, 1:2])
                    nc.vector.tensor_scalar_add(lam1, lam1, LAMBDA_INIT)
                    nc.gpsimd.partition_broadcast(lamc, lam1)
                    for n0 in range(0, T, 512):
                        nc.gpsimd.partition_broadcast(
                            padbc[:, n0 : n0 + 512], pad_sb[:, n0 : n0 + 512]
                        )

                # ============ phase 2 ============
                ATILE = 1024  # psA tile width (2 PSUM banks)
                with (
                    tc.tile_pool(name="psA", bufs=2, space="PSUM") as psA,
                    tc.tile_pool(name="sb2", bufs=2) as sb2,
                    tc.tile_pool(name="obufs", bufs=4) as obufs,
                    tc.tile_pool(name="rbufs", bufs=4) as rbufs,
                ):
                    def stage_a(j):
                        """Scores -> mask -> exp into u bands, both branches."""
                        q_lo = 0 if j == 0 else 128 * (j - 1)
                        w = widths[j]
                        for br in range(2):
                            KT = (k1t, k2t)[br]
                            QT = (q1t, q2t)[br]
                            for t0 in range(0, w, ATILE):
                                tw = min(ATILE, w - t0)
                                s_ps = psA.tile([128, ATILE], F32, tag="s")
                                for n0 in range(0, tw, 512):
                                    nw = min(512, tw - n0)
                                    nc.tensor.matmul(
                                        s_ps[:, n0 : n0 + nw],
                                        KT[:, j * 128 : (j + 1) * 128],
                                        QT[:, q_lo + t0 + n0 : q_lo + t0 + n0 + nw],
                                        start=True,
                                        stop=True,
                                    )
                                if t0 == 0:
                                    # superdiag+diag triangular mask (-> u = 1)
                                    if j == 0:
                                        mask_eng.tensor_mul(
                                            s_ps[:, 0:128], s_ps[:, 0:128], trid_sb
                                        )
                                    else:
                                        mask_eng.tensor_mul(
                                            s_ps[:, 0:256], s_ps[:, 0:256], trisd_sb
                                        )
                                nc.scalar.activation(
                                    u_band[br][
                                        :, band_off[j] + t0 : band_off[j] + t0 + tw
                                    ],
                                    s_ps[:, :tw],
                                    EXP,
                                    scale=SCALE,
                                )

                    # ---- V projection interleaved with stage A j=0..4 ----
                    with tc.tile_pool(name="ppV", bufs=1, space="PSUM") as ppV:
                        vps = ppV.tile([128, T], F32, tag="vproj")
                        vtb = obufs.tile([128, T], BF16, tag="vtb", bufs=1)
                        for ct in range(nC):
                            xt = xs.tile([128, T], BF16, tag="xt")
                            nc.sync.dma_start(xt, vT.ap()[ct * 128 : (ct + 1) * 128, :])
                            for n0 in range(0, T, 512):
                                nc.tensor.matmul(
                                    vps[:, n0 : n0 + 512],
                                    wv_w(ct),
                                    xt[:, n0 : n0 + 512],
                                    start=(ct == 0),
                                    stop=(ct == nC - 1),
                                )
                            if ct < 5 and not phase1_only:
                                stage_a(ct)
                        for ni, n0 in enumerate(range(0, T, 1024)):
                            if ni % 2 == 0:
                                nc.scalar.copy(
                                    vtb[:, n0 : n0 + 1024], vps[:, n0 : n0 + 1024]
                                )
                            else:
                                nc.vector.tensor_copy(
                                    vtb[:, n0 : n0 + 1024], vps[:, n0 : n0 + 1024]
                                )

                    # V natural blocks + per-block column sums + suffix sums
                    with tc.tile_pool(name="ppT", bufs=2, space="PSUM") as ppT:
                        for j in range(nT):
                            vtr = ppT.tile([128, 128], BF16, tag="m")
                            nc.tensor.transpose(
                                vtr, vtb[:, j * 128 : (j + 1) * 128], idb_w
                            )
                            if j % 2 == 0:
                                nc.vector.tensor_copy(vsb[:, j, :], vtr)
                            else:
                                nc.scalar.copy(vsb[:, j, :], vtr)
                    nc.vector.tensor_reduce(
                        vcols,
                        vtb.rearrange("p (j q) -> p j q", j=nT),
                        mybir.AxisListType.X,
                        mybir.AluOpType.add,
                    )
                    nc.vector.memset(sufv[:, nT - 1 : nT], 0.0)
                    nc.vector.memset(sufv[:, nT - 2 : nT - 1], 0.0)
                    for i in range(nT - 3, -1, -1):
                        nc.vector.tensor_add(
                            sufv[:, i : i + 1], sufv[:, i + 1 : i + 2],
                            vcols[:, i + 2 : i + 3],
                        )
                    nc.vector.tensor_add(tvn, sufv[:, 0:1], vcols[:, 0:1])
                    nc.vector.tensor_add(tvn, tvn, vcols[:, 1:2])
                    nc.scalar.mul(tvn, tvn, 1.0 / T)
                    # pad rows see uniform attention in both branches:
                    # out = (1 - lambda) * mean(V)
                    tvl = consts.tile([128, 1], F32, tag="tvl")
                    nc.vector.tensor_scalar(
                        tvl, tvn, lamc, None, mybir.AluOpType.mult
                    )
                    nc.vector.tensor_sub(tvn, tvn, tvl)

                    # ---- stage B + remaining stage A groups ----
                    pending = []

                    def emit_combine(q0, br_res):
                        # oT = O1*R1 - lambda*O2*R2; pad rows -> uniform value.
                        (ou1sb, r1bc), (ou2sb, r2bc) = br_res
                        c1 = sb2.tile([128, NQ], F32, tag="c1")
                        comb_eng.tensor_mul(c1, ou1sb, r1bc)
                        c2 = sb2.tile([128, NQ], F32, tag="c2")
                        nc.vector.scalar_tensor_tensor(
                            c2, ou2sb, lamc, r2bc,
                            mybir.AluOpType.mult, mybir.AluOpType.mult,
                        )
                        oT = sb2.tile([128, NQ], F32, tag="oT")
                        comb_eng.tensor_sub(oT, c1, c2)
                        oT2 = sb2.tile([128, NQ], F32, tag="oT2")
                        nc.vector.scalar_tensor_tensor(
                            oT2, padbc[:, q0 : q0 + NQ], tvn, oT,
                            mybir.AluOpType.mult, mybir.AluOpType.add,
                        )
                        nc.sync.dma_start(out.ap()[:, q0 : q0 + NQ], oT2)

                    def b_window(c, j):
                        q0 = c * NQ
                        q_lo = 0 if j == 0 else 128 * (j - 1)
                        oo = max(0, q_lo - q0)
                        wo = max(0, q0 - q_lo)
                        return q0, oo, wo, NQ - oo

                    class BChunk:
                        """One query chunk of stage B, emitted in j-slices so
                        the PE stream can alternate with stage-A groups."""

                        def __init__(self, c, psO, psD):
                            self.c = c
                            self.q0 = c * NQ
                            self.jmax = min((self.q0 + NQ) // 128, nT - 1)
                            if len(pending) > 1:
                                emit_combine(*pending.pop(0))
                            self.ou_h = [
                                psO.tile([128, NQ], F32, tag="ou", name=f"ou{br}")
                                for br in range(2)
                            ]
                            self.d2ps = psD.tile([128, NQ], F32, tag="dd")
                            # masked-count row seeds the denominator chain
                            for br in range(2):
                                nc.tensor.matmul(
                                    self.d2ps[32 * br : 32 * br + 1, :],
                                    onesb_w[0:1, :],
                                    sufb_sb[:, self.q0 : self.q0 + NQ],
                                    start=True,
                                    stop=False,
                                    skip_group_check=True,
                                )

                        def slice(self, js):
                            for j in js:
                                _, oo, wo, w = b_window(self.c, j)
                                for br in range(2):
                                    nc.tensor.matmul(
                                        self.ou_h[br][:, oo : oo + w],
                                        vsb[:, j, :],
                                        u_band[br][
                                            :, band_off[j] + wo : band_off[j] + wo + w
                                        ],
                                        start=(j == 0),
                                        stop=(j == self.jmax),
                                        skip_group_check=True,
                                    )
                            for br in range(2):
                                dps = self.d2ps[32 * br : 32 * br + 1, :]
                                for j in js:
                                    _, oo, wo, w = b_window(self.c, j)
                                    nc.tensor.matmul(
                                        dps[:, oo : oo + w],
                                        onesb_w,
                                        u_band[br][
                                            :, band_off[j] + wo : band_off[j] + wo + w
                                        ],
                                        start=False,
                                        stop=(j == self.jmax),
                                        skip_group_check=True,
                                    )

                        def epilogue(self):
                            # drain PSUM fast, defer the combine
                            q0 = self.q0
                            br_res = []
                            for br in range(2):
                                dps = self.d2ps[32 * br : 32 * br + 1, :]
                                rrow = sb2.tile([1, NQ], F32, tag="rrow")
                                nc.vector.reciprocal(rrow, dps)
                                # zero padded columns (pad handled in combine)
                                nc.vector.tensor_mul(
                                    rrow, rrow, npad_sb[:, q0 : q0 + NQ]
                                )
                                rbc = rbufs.tile([128, NQ], F32, tag="rbc")
                                nc.gpsimd.partition_broadcast(rbc, rrow)
                                # drain O_u, adding skipped-region suffix V sums
                                ousb = obufs.tile([128, NQ], F32, tag="ousb")
                                nb = NQ // 128
                                nc.vector.tensor_tensor(
                                    ousb.rearrange("p (b q) -> p b q", b=nb),
                                    self.ou_h[br].rearrange("p (b q) -> p b q", b=nb),
                                    sufv[:, q0 // 128 : q0 // 128 + nb]
                                    .unsqueeze(2)
                                    .broadcast_to([128, nb, 128]),
                                    mybir.AluOpType.add,
                                )
                                br_res.append((ousb, rbc))
                            pending.append((q0, br_res))

                    if not phase1_only:
                        with (
                            tc.tile_pool(name="psO", bufs=3, space="PSUM") as psO,
                            tc.tile_pool(name="psD", bufs=1, space="PSUM") as psD,
                        ):
                            # Fine-grained interleave: single A groups alternate
                            # with j-slices of the one active B chunk, so the
                            # in-order PE stream never sits long behind the ACT
                            # exp queue (psA is only 2 tiles deep).
                            B = lambda c: BChunk(c, psO, psD)
                            b = B(0)
                            stage_a(5)
                            stage_a(6)
                            b.slice(range(0, 3))
                            stage_a(7)
                            b.slice(range(3, 5))
                            b.epilogue()
                            stage_a(8)
                            b = B(1)
                            b.slice(range(0, 3))
                            stage_a(9)
                            b.slice(range(3, 6))
                            stage_a(10)
                            b.slice(range(6, 9))
                            b.epilogue()
                            stage_a(11)
                            b = B(2)
                            b.slice(range(0, 4))
                            stage_a(12)
                            b.slice(range(4, 8))
                            stage_a(13)
                            b.slice(range(8, 13))
                            b.epilogue()
                            stage_a(14)
                            stage_a(15)
                            b = B(3)
                            b.slice(range(0, 13))
                            b.slice(range(13, nT))
                            b.epilogue()
                            for args in pending:
                                emit_combine(*args)
                            pending = []

    nc.compile()
    return nc


def _host_constants(T, nT):
    kl = np.arange(128)[:, None]
    ql = np.arange(128)[None, :]
    trid = (kl <= ql + 1).astype(np.float32)
    tris = ((kl + 128) <= (ql + 1)).astype(np.float32)
    tiles = np.arange(T) // 128
    sufcnt = (128.0 * np.maximum(0, nT - 2 - tiles)).astype(np.float32)[None, :]
    return trid, tris, sufcnt


_NC_CACHE = {}


def make_in_maps(q, k, v, pad_mask, Wq, Wk, Wv, lq1, lk1, lq2, lk2):
    """Per-core input dicts (host-side sharding + layout marshaling)."""
    B, T, C = q.shape
    nT, nC = T // 128, C // 128
    bf16 = ml_dtypes.bfloat16
    trid, tris, sufcnt = _host_constants(T, nT)

    def blocked(W):
        # [C, D] -> [128, nC*D] with cols ct*D + d = W[ct*128 + p, d]
        D = W.shape[1]
        return (
            np.asarray(W)
            .reshape(nC, 128, D)
            .transpose(1, 0, 2)
            .reshape(128, nC * D)
        )

    cbK = np.ascontiguousarray(blocked(Wk)).astype(bf16)
    cbR = np.concatenate(
        [
            blocked(Wq),
            blocked(Wv),
            np.eye(128, dtype=np.float32),
            np.ones((128, 1), dtype=np.float32),
        ],
        axis=1,
    ).astype(bf16)
    lvec = np.stack(
        [np.asarray(lq1), np.asarray(lk1), np.asarray(lq2), np.asarray(lk2)], axis=1
    ).astype(np.float32)
    cbf32 = np.concatenate([tris, trid, lvec], axis=1).astype(np.float32)
    sufcnt_bf = np.ascontiguousarray(sufcnt.astype(bf16))

    in_maps = []
    for b in range(B):
        padf = np.asarray(pad_mask[b], dtype=np.float32)
        pn2 = np.ascontiguousarray(
            np.concatenate([padf, 1.0 - padf])[None, :].astype(np.float32)
        )
        in_maps.append(
            dict(
                qT=np.ascontiguousarray(np.asarray(q[b]).T.astype(bf16)),
                kT=np.ascontiguousarray(np.asarray(k[b]).T.astype(bf16)),
                vT=np.ascontiguousarray(np.asarray(v[b]).T.astype(bf16)),
                cbK=cbK,
                cbR=cbR,
                cbf32=cbf32,
                pn2=pn2,
                sufcnt_bf=sufcnt_bf,
            )
        )
    return in_maps


def kernel(q, k, v, pad_mask, Wq, Wk, Wv, lq1, lk1, lq2, lk2):
    B, T, C = q.shape
    assert B == N_CORES
    key = (T, C)
    if key not in _NC_CACHE:
        _NC_CACHE[key] = build_nc(T=T, C=C)
    nc = _NC_CACHE[key]
    in_maps = make_in_maps(q, k, v, pad_mask, Wq, Wk, Wv, lq1, lk1, lq2, lk2)
    res = run_bass_kernel_spmd(nc, in_maps, core_ids=list(range(N_CORES)))
    return np.stack(
        [np.ascontiguousarray(r["out"].T) for r in res.results], axis=0
    )


# revision 17
# speedup vs baseline: 8.6017x; 7.2965x over previous
"""DiffHead (differential attention head) Trainium2 Bass kernel.

Strategy (hardcoded for B=8, T=2048, C=1024, HS=128, 8 cores):
  - Data-parallel over batch: one batch element per NeuronCore.
  - Host side only reshapes/shards: per-core q/k/v slices are passed
    transposed ([C, T]) so the projection matmuls contract over C on the
    partition axis. All FLOPs run on device.
  - Scores are computed transposed (S^T[k, q]); masked fills of 1e-9
    scale to exactly 1.0f after exp, so the fully-masked region beyond
    the diagonal/superdiagonal blocks is never computed: its
    contributions are closed-form (suffix sums of V rows + a masked
    count folded into the denominator matmul chain).
  - Phase 2 is split into stage A (scores -> mask -> exp into an SBUF
    "u band" per branch) and stage B (U@V + denominator matmuls +
    combine), manually interleaved so the PE never waits long on the
    ACT exp stream. The V projection itself interleaves with the first
    stage-A group to hide its DMA pacing.
  - All small constants + weights ship in packed blobs and inputs load
    one DMA per 128-row tile, keeping HWDGE descriptor generation off
    the critical path.
  - Row->all-partition broadcasts (reciprocal rows, pad row, lambda)
    and part of the combine run on the otherwise idle GpSimd engine.
"""

import numpy as np
import ml_dtypes

try:
    import concourse.bacc as bacc
except ImportError:  # pragma: no cover
    import sys

    sys.path.insert(0, "/opt/trn_rl_repo")
    import concourse.bacc as bacc

import concourse.mybir as mybir
import concourse.tile as tile
from concourse.bass_utils import run_bass_kernel_spmd

F32 = mybir.dt.float32
F32R = mybir.dt.float32r
BF16 = mybir.dt.bfloat16
EXP = mybir.ActivationFunctionType.Exp

HS = 128
LAMBDA_INIT = 0.8
N_CORES = 8

# feature flags (fallbacks if an op turns out unsupported)
MASK_ON_POOL = False
COMBINE_ON_POOL = True


def _r(ap):
    """View an f32 AP as float32r so the PE runs at full rate."""
    return ap.bitcast(F32R)


def _band_widths(T, nT):
    """Score-band column width per k-tile j: queries q >= 128*(j-1)."""
    return [T - (0 if j == 0 else 128 * (j - 1)) for j in range(nT)]


def build_nc(T=2048, C=1024, NQ=512, repeat=1, phase1_only=False):
    """Build the per-core Bass program. Same NEFF on all 8 cores (SPMD).

    repeat > 1 wraps the body in a hardware loop (for wall-clock slope
    timing); results are identical since the body is idempotent.
    """
    import contextlib

    nT = T // 128
    nC = C // 128
    NQ = min(NQ, T)
    SCALE = float(HS) ** -0.5

    widths = _band_widths(T, nT)
    band_off = [0] * nT
    for j in range(1, nT):
        band_off[j] = band_off[j - 1] + widths[j - 1]
    band_cols = band_off[-1] + widths[-1]

    # K weights ship alone (needed first); the rest packs into cbR:
    # [wq | wv | idb | onesb]
    KCOLS = nC * 2 * HS
    WQ0 = 0
    WV0 = WQ0 + nC * 2 * HS
    IDB0 = WV0 + nC * HS
    ONE0 = IDB0 + 128
    RCOLS = ONE0 + 1
    # packed f32 blob offsets: [tris | trid | lvec]
    TRIS0, TRID0, LV0 = 0, 128, 256
    F32COLS = LV0 + 4

    nc = bacc.Bacc("TRN2", target_bir_lowering=False, num_devices=N_CORES)

    qT = nc.dram_tensor("qT", [C, T], BF16, kind="ExternalInput")
    kT = nc.dram_tensor("kT", [C, T], BF16, kind="ExternalInput")
    vT = nc.dram_tensor("vT", [C, T], BF16, kind="ExternalInput")
    cbK = nc.dram_tensor("cbK", [128, KCOLS], BF16, kind="ExternalInput")
    cbR = nc.dram_tensor("cbR", [128, RCOLS], BF16, kind="ExternalInput")
    cbf32 = nc.dram_tensor("cbf32", [128, F32COLS], F32, kind="ExternalInput")
    pn2 = nc.dram_tensor("pn2", [1, 2 * T], BF16, kind="ExternalInput")
    sufcnt_bf = nc.dram_tensor("sufcnt_bf", [1, T], BF16, kind="ExternalInput")
    # output stays transposed ([dv, T]); the host un-transposes.
    out = nc.dram_tensor("out", [HS, T], F32, kind="ExternalOutput")

    with tile.TileContext(nc) as tc:
        rep_cm = tc.For_i(0, repeat, 1) if repeat > 1 else contextlib.nullcontext()
        with (
            rep_cm,
            tc.tile_pool(name="consts", bufs=1) as consts,
            tc.tile_pool(name="persist", bufs=1) as persist,
        ):
            # ---- packed constants (K weights first: needed immediately) ----
            cbk_sb = consts.tile([128, KCOLS], BF16, tag="cbK")
            nc.sync.dma_start(cbk_sb, cbK.ap())
            cbr_sb = consts.tile([128, RCOLS], BF16, tag="cbR")
            cf = consts.tile([128, F32COLS], F32, tag="cbf32")
            pn_sb = consts.tile([1, 2 * T], BF16, tag="pn2")
            sufb_sb = consts.tile([1, T], BF16, tag="sufb")

            def wk_w(ct, h):
                return cbk_sb[:, ct * 2 * HS + h * HS : ct * 2 * HS + (h + 1) * HS]

            def wq_w(ct, h):
                return cbr_sb[:, WQ0 + ct * 2 * HS + h * HS : WQ0 + ct * 2 * HS + (h + 1) * HS]

            def wv_w(ct):
                return cbr_sb[:, WV0 + ct * HS : WV0 + (ct + 1) * HS]

            idb_w = cbr_sb[:, IDB0 : IDB0 + 128]
            onesb_w = cbr_sb[:, ONE0 : ONE0 + 1]
            trisd_sb = cf[:, TRIS0 : TRIS0 + 256]
            trid_sb = cf[:, TRID0 : TRID0 + 128]
            lv_sb = cf[:, LV0 : LV0 + 4]
            pad_sb = pn_sb[:, 0:T]
            npad_sb = pn_sb[:, T : 2 * T]

            # ---- persistent intermediates ----
            q1t = persist.tile([128, T], F32R, tag="q1t")
            q2t = persist.tile([128, T], F32R, tag="q2t")
            k1t = persist.tile([128, T], F32R, tag="k1t")
            k2t = persist.tile([128, T], F32R, tag="k2t")
            vsb = persist.tile([128, nT, 128], BF16, tag="vsb")  # V natural
            padbc = persist.tile([128, T], BF16, tag="padbc")
            vcols = persist.tile([128, nT], F32, tag="vcols")
            sufv = persist.tile([128, nT], F32, tag="sufv")
            tvn = persist.tile([128, 1], F32, tag="tvn")  # (1-lambda)*mean(V)
            lamc = persist.tile([128, 1], F32, tag="lamc")
            u_band = [
                persist.tile([128, band_cols], BF16, tag="u1", name="u1"),
                persist.tile([128, band_cols], BF16, tag="u2", name="u2"),
            ]

            mask_eng = nc.gpsimd if MASK_ON_POOL else nc.vector
            comb_eng = nc.gpsimd if COMBINE_ON_POOL else nc.vector

            with tc.tile_pool(name="xs", bufs=6) as xs:
                # ============ K and Q projections (full-PSUM scope) ============
                with tc.tile_pool(name="ppK", bufs=2, space="PSUM") as ppK:
                    first = True
                    for xdram, w_of, outs in (
                        (kT, wk_w, (k1t, k2t)),
                        (qT, wq_w, (q1t, q2t)),
                    ):
                        ps = [
                            ppK.tile([128, T], F32, tag="proj", name=f"ps{h}")
                            for h in range(2)
                        ]
                        for ct in range(nC):
                            xt = xs.tile([128, T], BF16, tag="xt")
                            nc.sync.dma_start(
                                xt, xdram.ap()[ct * 128 : (ct + 1) * 128, :]
                            )
                            if first:
                                # remaining consts ride behind the first tile
                                nc.sync.dma_start(cbr_sb, cbR.ap())
                                nc.sync.dma_start(cf, cbf32.ap())
                                nc.sync.dma_start(pn_sb, pn2.ap())
                                nc.sync.dma_start(sufb_sb, sufcnt_bf.ap())
                                first = False
                            for h in range(2):
                                for n0 in range(0, T, 512):
                                    nc.tensor.matmul(
                                        ps[h][:, n0 : n0 + 512],
                                        w_of(ct, h),
                                        xt[:, n0 : n0 + 512],
                                        start=(ct == 0),
                                        stop=(ct == nC - 1),
                                    )
                        for h in range(2):
                            for ni, n0 in enumerate(range(0, T, 1024)):
                                if (h + ni) % 2 == 0:
                                    nc.scalar.copy(
                                        outs[h][:, n0 : n0 + 1024],
                                        ps[h][:, n0 : n0 + 1024],
                                    )
                                else:
                                    nc.vector.tensor_copy(
                                        outs[h][:, n0 : n0 + 1024],
                                        ps[h][:, n0 : n0 + 1024],
                                    )

                # ============ lambda + pad broadcast (tiny PSUM scope) ============
                with tc.tile_pool(name="ppL", bufs=1, space="PSUM") as ppL:
                    dots_ps = ppL.tile([1, 2], F32, tag="t")
                    nc.tensor.matmul(
                        dots_ps[:, 0:1], lv_sb[:, 0:1], lv_sb[:, 1:2],
                        start=True, stop=True,
                    )
                    nc.tensor.matmul(
                        dots_ps[:, 1:2], lv_sb[:, 2:3], lv_sb[:, 3:4],
                        start=True, stop=True,
                    )
                    eexp = consts.tile([1, 2], F32, tag="eexp")
                    nc.scalar.activation(eexp, dots_ps, EXP)
                    lam1 = consts.tile([1, 1], F32, tag="lam1")
                    nc.vector.tensor_sub(lam1, eexp[:, 0:1], eexp[:, 1:2])
                    nc.vector.tensor_scalar_add(lam1, lam1, LAMBDA_INIT)
                    nc.gpsimd.partition_broadcast(lamc, lam1)
                    for n0 in range(0, T, 512):
                        nc.gpsimd.partition_broadcast(
                            padbc[:, n0 : n0 + 512], pad_sb[:, n0 : n0 + 512]
                        )

                # ============ phase 2 ============
                ATILE = 1024  # psA tile width (2 PSUM banks)
                with (
                    tc.tile_pool(name="psA", bufs=2, space="PSUM") as psA,
                    tc.tile_pool(name="sb2", bufs=2) as sb2,
                    tc.tile_pool(name="obufs", bufs=4) as obufs,
                    tc.tile_pool(name="rbufs", bufs=4) as rbufs,
                ):
                    def stage_a(j):
                        """Scores -> mask -> exp into u bands, both branches."""
                        q_lo = 0 if j == 0 else 128 * (j - 1)
                        w = widths[j]
                        for br in range(2):
                            KT = (k1t, k2t)[br]
                            QT = (q1t, q2t)[br]
                            for t0 in range(0, w, ATILE):
                                tw = min(ATILE, w - t0)
                                s_ps = psA.tile([128, ATILE], F32, tag="s")
                                for n0 in range(0, tw, 512):
                                    nw = min(512, tw - n0)
                                    nc.tensor.matmul(
                                        s_ps[:, n0 : n0 + nw],
                                        KT[:, j * 128 : (j + 1) * 128],
                                        QT[:, q_lo + t0 + n0 : q_lo + t0 + n0 + nw],
                                        start=True,
                                        stop=True,
                                    )
                                if t0 == 0:
                                    # superdiag+diag triangular mask (-> u = 1)
                                    if j == 0:
                                        mask_eng.tensor_mul(
                                            s_ps[:, 0:128], s_ps[:, 0:128], trid_sb
                                        )
                                    else:
                                        mask_eng.tensor_mul(
                                            s_ps[:, 0:256], s_ps[:, 0:256], trisd_sb
                                        )
                                nc.scalar.activation(
                                    u_band[br][
                                        :, band_off[j] + t0 : band_off[j] + t0 + tw
                                    ],
                                    s_ps[:, :tw],
                                    EXP,
                                    scale=SCALE,
                                )

                    # ---- V projection interleaved with stage A j=0..4 ----
                    with tc.tile_pool(name="ppV", bufs=1, space="PSUM") as ppV:
                        vps = ppV.tile([128, T], F32, tag="vproj")
                        vtb = obufs.tile([128, T], BF16, tag="vtb", bufs=1)
                        for ct in range(nC):
                            xt = xs.tile([128, T], BF16, tag="xt")
                            nc.sync.dma_start(xt, vT.ap()[ct * 128 : (ct + 1) * 128, :])
                            for n0 in range(0, T, 512):
                                nc.tensor.matmul(
                                    vps[:, n0 : n0 + 512],
                                    wv_w(ct),
                                    xt[:, n0 : n0 + 512],
                                    start=(ct == 0),
                                    stop=(ct == nC - 1),
                                )
                            if ct < 5 and not phase1_only:
                                stage_a(ct)
                        for n0 in range(0, T, 1024):
                            nc.vector.tensor_copy(
                                vtb[:, n0 : n0 + 1024], vps[:, n0 : n0 + 1024]
                            )

                    # V natural blocks + per-block column sums + suffix sums
                    with tc.tile_pool(name="ppT", bufs=2, space="PSUM") as ppT:
                        for j in range(nT):
                            vtr = ppT.tile([128, 128], BF16, tag="m")
                            nc.tensor.transpose(
                                vtr, vtb[:, j * 128 : (j + 1) * 128], idb_w
                            )
                            nc.vector.tensor_copy(vsb[:, j, :], vtr)
                    nc.vector.tensor_reduce(
                        vcols,
                        vtb.rearrange("p (j q) -> p j q", j=nT),
                        mybir.AxisListType.X,
                        mybir.AluOpType.add,
                    )
                    nc.vector.memset(sufv[:, nT - 1 : nT], 0.0)
                    nc.vector.memset(sufv[:, nT - 2 : nT - 1], 0.0)
                    for i in range(nT - 3, -1, -1):
                        nc.vector.tensor_add(
                            sufv[:, i : i + 1], sufv[:, i + 1 : i + 2],
                            vcols[:, i + 2 : i + 3],
                        )
                    nc.vector.tensor_add(tvn, sufv[:, 0:1], vcols[:, 0:1])
                    nc.vector.tensor_add(tvn, tvn, vcols[:, 1:2])
                    nc.vector.tensor_scalar_mul(tvn, tvn, 1.0 / T)
                    # pad rows see uniform attention in both branches:
                    # out = (1 - lambda) * mean(V)
                    tvl = consts.tile([128, 1], F32, tag="tvl")
                    nc.vector.tensor_scalar(
                        tvl, tvn, lamc, None, mybir.AluOpType.mult
                    )
                    nc.vector.tensor_sub(tvn, tvn, tvl)

                    # ---- stage B + remaining stage A groups ----
                    pending = []

                    def emit_combine(q0, br_res, last=False):
                        # oT = O1*R1 - lambda*O2*R2; pad rows -> uniform value.
                        # The final chunk's combine skips the Pool handoff: PE
                        # is done, so the shortest serial chain wins.
                        ceng = nc.vector if last else comb_eng
                        (ou1sb, r1bc), (ou2sb, r2bc) = br_res
                        c1 = sb2.tile([128, NQ], F32, tag="c1")
                        ceng.tensor_mul(c1, ou1sb, r1bc)
                        c2 = sb2.tile([128, NQ], F32, tag="c2")
                        nc.vector.scalar_tensor_tensor(
                            c2, ou2sb, lamc, r2bc,
                            mybir.AluOpType.mult, mybir.AluOpType.mult,
                        )
                        oT = sb2.tile([128, NQ], F32, tag="oT")
                        ceng.tensor_sub(oT, c1, c2)
                        oT2 = sb2.tile([128, NQ], F32, tag="oT2")
                        nc.vector.scalar_tensor_tensor(
                            oT2, padbc[:, q0 : q0 + NQ], tvn, oT,
                            mybir.AluOpType.mult, mybir.AluOpType.add,
                        )
                        nc.sync.dma_start(out.ap()[:, q0 : q0 + NQ], oT2)

                    def b_window(c, j):
                        q0 = c * NQ
                        q_lo = 0 if j == 0 else 128 * (j - 1)
                        oo = max(0, q_lo - q0)
                        wo = max(0, q0 - q_lo)
                        return q0, oo, wo, NQ - oo

                    class BChunk:
                        """One query chunk of stage B, emitted in j-slices so
                        the PE stream can alternate with stage-A groups."""

                        def __init__(self, c, psO, psD):
                            self.c = c
                            self.q0 = c * NQ
                            self.jmax = min((self.q0 + NQ) // 128, nT - 1)
                            if pending:
                                emit_combine(*pending.pop(0))
                            self.ou_h = [
                                psO.tile([128, NQ], F32, tag="ou", name=f"ou{br}")
                                for br in range(2)
                            ]
                            self.d2ps = psD.tile([128, NQ], F32, tag="dd")
                            # masked-count row seeds the denominator chain
                            for br in range(2):
                                nc.tensor.matmul(
                                    self.d2ps[32 * br : 32 * br + 1, :],
                                    onesb_w[0:1, :],
                                    sufb_sb[:, self.q0 : self.q0 + NQ],
                                    start=True,
                                    stop=False,
                                    skip_group_check=True,
                                )

                        def dsum_slice(self, js):
                            for br in range(2):
                                dps = self.d2ps[32 * br : 32 * br + 1, :]
                                for j in js:
                                    _, oo, wo, w = b_window(self.c, j)
                                    nc.tensor.matmul(
                                        dps[:, oo : oo + w],
                                        onesb_w,
                                        u_band[br][
                                            :, band_off[j] + wo : band_off[j] + wo + w
                                        ],
                                        start=False,
                                        stop=(j == self.jmax),
                                        skip_group_check=True,
                                    )

                        def uv_slice(self, js):
                            for j in js:
                                _, oo, wo, w = b_window(self.c, j)
                                for br in range(2):
                                    nc.tensor.matmul(
                                        self.ou_h[br][:, oo : oo + w],
                                        vsb[:, j, :],
                                        u_band[br][
                                            :, band_off[j] + wo : band_off[j] + wo + w
                                        ],
                                        start=(j == 0),
                                        stop=(j == self.jmax),
                                        skip_group_check=True,
                                    )

                        def slice(self, js, last=False):
                            # denominators first: on the last slice their
                            # reciprocal/broadcast chain then overlaps the
                            # remaining U@V matmuls
                            self.dsum_slice(js)
                            if last:
                                self.epi_d()
                            self.uv_slice(js)

                        def epi_d(self):
                            q0 = self.q0
                            self.rbcs = []
                            for br in range(2):
                                dps = self.d2ps[32 * br : 32 * br + 1, :]
                                rrow = sb2.tile([1, NQ], F32, tag="rrow")
                                nc.vector.reciprocal(rrow, dps)
                                # zero padded columns (pad handled in combine)
                                nc.vector.tensor_mul(
                                    rrow, rrow, npad_sb[:, q0 : q0 + NQ]
                                )
                                rbc = rbufs.tile([128, NQ], F32, tag="rbc")
                                nc.gpsimd.partition_broadcast(rbc, rrow)
                                self.rbcs.append(rbc)

                        def epilogue(self):
                            # drain O_u (adding skipped-region suffix V sums),
                            # defer the combine
                            q0 = self.q0
                            br_res = []
                            for br in range(2):
                                ousb = obufs.tile([128, NQ], F32, tag="ousb")
                                nb = NQ // 128
                                nc.vector.tensor_tensor(
                                    ousb.rearrange("p (b q) -> p b q", b=nb),
                                    self.ou_h[br].rearrange("p (b q) -> p b q", b=nb),
                                    sufv[:, q0 // 128 : q0 // 128 + nb]
                                    .unsqueeze(2)
                                    .broadcast_to([128, nb, 128]),
                                    mybir.AluOpType.add,
                                )
                                br_res.append((ousb, self.rbcs[br]))
                            pending.append((q0, br_res))

                    if not phase1_only:
                        with (
                            tc.tile_pool(name="psO", bufs=3, space="PSUM") as psO,
                            tc.tile_pool(name="psD", bufs=1, space="PSUM") as psD,
                        ):
                            # Fine-grained interleave: single A groups alternate
                            # with j-slices of the one active B chunk, so the
                            # in-order PE stream never sits long behind the ACT
                            # exp queue (psA is only 2 tiles deep).
                            B = lambda c: BChunk(c, psO, psD)
                            b = B(0)
                            stage_a(5)
                            stage_a(6)
                            b.slice(range(0, 3))
                            stage_a(7)
                            b.slice(range(3, 5), last=True)
                            b.epilogue()
                            stage_a(8)
                            b = B(1)
                            b.slice(range(0, 3))
                            stage_a(9)
                            b.slice(range(3, 6))
                            stage_a(10)
                            b.slice(range(6, 9), last=True)
                            b.epilogue()
                            stage_a(11)
                            b = B(2)
                            b.slice(range(0, 4))
                            stage_a(12)
                            b.slice(range(4, 8))
                            stage_a(13)
                            b.slice(range(8, 13), last=True)
                            b.epilogue()
                            stage_a(14)
                            stage_a(15)
                            b = B(3)
                            b.slice(range(0, 13))
                            b.slice(range(13, nT), last=True)
                            b.epilogue()
                            for args in pending:
                                emit_combine(*args, last=True)
                            pending = []

    nc.compile()
    return nc


def _host_constants(T, nT):
    kl = np.arange(128)[:, None]
    ql = np.arange(128)[None, :]
    trid = (kl <= ql + 1).astype(np.float32)
    tris = ((kl + 128) <= (ql + 1)).astype(np.float32)
    tiles = np.arange(T) // 128
    sufcnt = (128.0 * np.maximum(0, nT - 2 - tiles)).astype(np.float32)[None, :]
    return trid, tris, sufcnt


_NC_CACHE = {}


def make_in_maps(q, k, v, pad_mask, Wq, Wk, Wv, lq1, lk1, lq2, lk2):
    """Per-core input dicts (host-side sharding + layout marshaling)."""
    B, T, C = q.shape
    nT, nC = T // 128, C // 128
    bf16 = ml_dtypes.bfloat16
    trid, tris, sufcnt = _host_constants(T, nT)

    def blocked(W):
        # [C, D] -> [128, nC*D] with cols ct*D + d = W[ct*128 + p, d]
        D = W.shape[1]
        return (
            np.asarray(W)
            .reshape(nC, 128, D)
            .transpose(1, 0, 2)
            .reshape(128, nC * D)
        )

    cbK = np.ascontiguousarray(blocked(Wk)).astype(bf16)
    cbR = np.concatenate(
        [
            blocked(Wq),
            blocked(Wv),
            np.eye(128, dtype=np.float32),
            np.ones((128, 1), dtype=np.float32),
        ],
        axis=1,
    ).astype(bf16)
    lvec = np.stack(
        [np.asarray(lq1), np.asarray(lk1), np.asarray(lq2), np.asarray(lk2)], axis=1
    ).astype(np.float32)
    cbf32 = np.concatenate([tris, trid, lvec], axis=1).astype(np.float32)
    sufcnt_bf = np.ascontiguousarray(sufcnt.astype(bf16))

    in_maps = []
    for b in range(B):
        padf = np.asarray(pad_mask[b], dtype=np.float32)
        pn2 = np.ascontiguousarray(
            np.concatenate([padf, 1.0 - padf])[None, :].astype(bf16)
        )
        in_maps.append(
            dict(
                qT=np.ascontiguousarray(np.asarray(q[b]).T.astype(bf16)),
                kT=np.ascontiguousarray(np.asarray(k[b]).T.astype(bf16)),
                vT=np.ascontiguousarray(np.asarray(v[b]).T.astype(bf16)),
                cbK=cbK,
                cbR=cbR,
                cbf32=cbf32,
                pn2=pn2,
                sufcnt_bf=sufcnt_bf,
            )
        )
    return in_maps


def kernel(q, k, v, pad_mask, Wq, Wk, Wv, lq1, lk1, lq2, lk2):
    B, T, C = q.shape
    assert B == N_CORES
    key = (T, C)
    if key not in _NC_CACHE:
        _NC_CACHE[key] = build_nc(T=T, C=C)
    nc = _NC_CACHE[key]
    in_maps = make_in_maps(q, k, v, pad_mask, Wq, Wk, Wv, lq1, lk1, lq2, lk2)
    res = run_bass_kernel_spmd(nc, in_maps, core_ids=list(range(N_CORES)))
    return np.stack(
        [np.ascontiguousarray(r["out"].T) for r in res.results], axis=0
    )


# revision 22
# speedup vs baseline: 10.2572x; 1.1925x over previous
"""DiffHead (differential attention head) Trainium2 Bass kernel.

Strategy (hardcoded for B=8, T=2048, C=1024, HS=128, 8 cores):
  - Data-parallel over batch: one batch element per NeuronCore.
  - Host side only reshapes/shards: per-core q/k/v slices are passed
    transposed ([C, T]) so the projection matmuls contract over C on the
    partition axis. All FLOPs run on device.
  - Scores are computed transposed (S^T[k, q]); masked fills of 1e-9
    scale to exactly 1.0f after exp, so the fully-masked region beyond
    the diagonal/superdiagonal blocks is never computed: its
    contributions are closed-form (suffix sums of V rows + a masked
    count folded into the denominator matmul chain).
  - Phase 2 is split into stage A (scores -> mask -> exp into an SBUF
    "u band" per branch) and stage B (U@V + denominator matmuls +
    combine), manually interleaved so the PE never waits long on the
    ACT exp stream. The V projection itself interleaves with the first
    stage-A group to hide its DMA pacing.
  - All small constants + weights ship in packed blobs and inputs load
    one DMA per 128-row tile, keeping HWDGE descriptor generation off
    the critical path.
  - Row->all-partition broadcasts (reciprocal rows, pad row, lambda)
    and part of the combine run on the otherwise idle GpSimd engine.
"""

import numpy as np
import ml_dtypes

try:
    import concourse.bacc as bacc
except ImportError:  # pragma: no cover
    import sys

    sys.path.insert(0, "/opt/trn_rl_repo")
    import concourse.bacc as bacc

import concourse.mybir as mybir
import concourse.tile as tile
from concourse.bass_utils import run_bass_kernel_spmd

F32 = mybir.dt.float32
F32R = mybir.dt.float32r
BF16 = mybir.dt.bfloat16
EXP = mybir.ActivationFunctionType.Exp

HS = 128
LAMBDA_INIT = 0.8
N_CORES = 8

# feature flags (fallbacks if an op turns out unsupported)
MASK_ON_POOL = False
COMBINE_ON_POOL = False


def _r(ap):
    """View an f32 AP as float32r so the PE runs at full rate."""
    return ap.bitcast(F32R)


def _band_widths(T, nT):
    """Score-band column width per k-tile j: queries q >= 128*(j-1)."""
    return [T - (0 if j == 0 else 128 * (j - 1)) for j in range(nT)]


def build_nc(T=2048, C=1024, NQ=512, repeat=1, phase1_only=False):
    """Build the per-core Bass program. Same NEFF on all 8 cores (SPMD).

    repeat > 1 wraps the body in a hardware loop (for wall-clock slope
    timing); results are identical since the body is idempotent.
    """
    import contextlib

    nT = T // 128
    nC = C // 128
    NQ = min(NQ, T)
    SCALE = float(HS) ** -0.5

    widths = _band_widths(T, nT)
    band_off = [0] * nT
    for j in range(1, nT):
        band_off[j] = band_off[j - 1] + widths[j - 1]
    band_cols = band_off[-1] + widths[-1]

    # K weights ship alone (needed first); the rest packs into cbR:
    # [wq | wv | idb | onesb]
    KCOLS = nC * 2 * HS
    WQ0 = 0
    WV0 = WQ0 + nC * 2 * HS
    IDB0 = WV0 + nC * HS
    ONE0 = IDB0 + 128
    ONESR0 = ONE0 + 1
    RCOLS = ONESR0 + 128
    # packed f32 blob offsets: [tris | trid | lvec | tiled lq1 | tiled lq2]
    TRIS0, TRID0, LV0 = 0, 128, 256
    LQ1B0 = LV0 + 4
    LQ2B0 = LQ1B0 + 128
    F32COLS = LQ2B0 + 128

    nc = bacc.Bacc("TRN2", target_bir_lowering=False, num_devices=N_CORES)

    qT = nc.dram_tensor("qT", [C, T], BF16, kind="ExternalInput")
    kT = nc.dram_tensor("kT", [C, T], BF16, kind="ExternalInput")
    vT = nc.dram_tensor("vT", [C, T], BF16, kind="ExternalInput")
    cbK = nc.dram_tensor("cbK", [128, KCOLS], BF16, kind="ExternalInput")
    cbR = nc.dram_tensor("cbR", [128, RCOLS], BF16, kind="ExternalInput")
    cbf32 = nc.dram_tensor("cbf32", [128, F32COLS], F32, kind="ExternalInput")
    pn2 = nc.dram_tensor("pn2", [1, 2 * T], BF16, kind="ExternalInput")
    padbc128 = nc.dram_tensor("padbc128", [128, T], BF16, kind="ExternalInput")
    sufcnt_bf = nc.dram_tensor("sufcnt_bf", [1, T], BF16, kind="ExternalInput")
    # output stays transposed ([dv, T]); the host un-transposes.
    out = nc.dram_tensor("out", [HS, T], F32, kind="ExternalOutput")

    with tile.TileContext(nc) as tc:
        rep_cm = tc.For_i(0, repeat, 1) if repeat > 1 else contextlib.nullcontext()
        with (
            rep_cm,
            tc.tile_pool(name="consts", bufs=1) as consts,
            tc.tile_pool(name="persist", bufs=1) as persist,
        ):
            # ---- packed constants (K weights first: needed immediately) ----
            cbk_sb = consts.tile([128, KCOLS], BF16, tag="cbK")
            nc.sync.dma_start(cbk_sb, cbK.ap())
            cbr_sb = consts.tile([128, RCOLS], BF16, tag="cbR")
            cf = consts.tile([128, F32COLS], F32, tag="cbf32")
            pn_sb = consts.tile([1, 2 * T], BF16, tag="pn2")
            sufb_sb = consts.tile([1, T], BF16, tag="sufb")

            def wk_w(ct, h):
                return cbk_sb[:, ct * 2 * HS + h * HS : ct * 2 * HS + (h + 1) * HS]

            def wq_w(ct, h):
                return cbr_sb[:, WQ0 + ct * 2 * HS + h * HS : WQ0 + ct * 2 * HS + (h + 1) * HS]

            def wv_w(ct):
                return cbr_sb[:, WV0 + ct * HS : WV0 + (ct + 1) * HS]

            idb_w = cbr_sb[:, IDB0 : IDB0 + 128]
            onesb_w = cbr_sb[:, ONE0 : ONE0 + 1]
            onesr_bw = cbr_sb[0:1, ONESR0 : ONESR0 + 128]
            lq1b_w = cf[:, LQ1B0 : LQ1B0 + 128]
            lq2b_w = cf[:, LQ2B0 : LQ2B0 + 128]
            trisd_sb = cf[:, TRIS0 : TRIS0 + 256]
            trid_sb = cf[:, TRID0 : TRID0 + 128]
            lv_sb = cf[:, LV0 : LV0 + 4]
            pad_sb = pn_sb[:, 0:T]
            npad_sb = pn_sb[:, T : 2 * T]

            # ---- persistent intermediates ----
            q1t = persist.tile([128, T], F32R, tag="q1t")
            q2t = persist.tile([128, T], F32R, tag="q2t")
            k1t = persist.tile([128, T], F32R, tag="k1t")
            k2t = persist.tile([128, T], F32R, tag="k2t")
            vsb = persist.tile([128, nT, 128], BF16, tag="vsb")  # V natural
            padbc = persist.tile([128, T], BF16, tag="padbc")
            vcols = persist.tile([128, nT], F32, tag="vcols")
            sufv = persist.tile([128, nT], F32, tag="sufv")
            tvn = persist.tile([128, 1], F32, tag="tvn")  # (1-lambda)*mean(V)
            lamc = persist.tile([128, 1], F32, tag="lamc")
            u_band = [
                persist.tile([128, band_cols], BF16, tag="u1", name="u1"),
                persist.tile([128, band_cols], BF16, tag="u2", name="u2"),
            ]

            mask_eng = nc.gpsimd if MASK_ON_POOL else nc.vector
            comb_eng = nc.gpsimd if COMBINE_ON_POOL else nc.vector

            with tc.tile_pool(name="xs", bufs=6) as xs:
                # ============ K and Q projections (full-PSUM scope) ============
                with tc.tile_pool(name="ppK", bufs=2, space="PSUM") as ppK:
                    first = True
                    for xdram, w_of, outs in (
                        (kT, wk_w, (k1t, k2t)),
                        (qT, wq_w, (q1t, q2t)),
                    ):
                        ps = [
                            ppK.tile([128, T], F32, tag="proj", name=f"ps{h}")
                            for h in range(2)
                        ]
                        for ct in range(nC):
                            xt = xs.tile([128, T], BF16, tag="xt")
                            nc.sync.dma_start(
                                xt, xdram.ap()[ct * 128 : (ct + 1) * 128, :]
                            )
                            if first:
                                # remaining consts ride behind the first tile
                                nc.sync.dma_start(cbr_sb, cbR.ap())
                                nc.sync.dma_start(cf, cbf32.ap())
                                nc.sync.dma_start(pn_sb, pn2.ap())
                                nc.sync.dma_start(sufb_sb, sufcnt_bf.ap())
                                first = False
                            for h in range(2):
                                for n0 in range(0, T, 512):
                                    nc.tensor.matmul(
                                        ps[h][:, n0 : n0 + 512],
                                        w_of(ct, h),
                                        xt[:, n0 : n0 + 512],
                                        start=(ct == 0),
                                        stop=(ct == nC - 1),
                                    )
                                if ct == nC - 1:
                                    # drain each half right behind its stop so
                                    # the next tensor's PSUM frees sooner
                                    for ni, n0 in enumerate(range(0, T, 1024)):
                                        if (h + ni) % 2 == 0:
                                            nc.scalar.copy(
                                                outs[h][:, n0 : n0 + 1024],
                                                ps[h][:, n0 : n0 + 1024],
                                            )
                                        else:
                                            nc.vector.tensor_copy(
                                                outs[h][:, n0 : n0 + 1024],
                                                ps[h][:, n0 : n0 + 1024],
                                            )

                # ============ lambda (broadcast via host-tiled weights) ============
                with tc.tile_pool(name="ppL", bufs=1, space="PSUM") as ppL:
                    # dots_bc[p, i] = sum_d lqi[d] * lki[d] for every p: the
                    # weights are lq1/lq2 replicated across columns on host
                    dots_ps = ppL.tile([128, 2], F32, tag="t")
                    nc.tensor.matmul(
                        dots_ps[:, 0:1], lq1b_w, lv_sb[:, 1:2],
                        start=True, stop=True,
                    )
                    nc.tensor.matmul(
                        dots_ps[:, 1:2], lq2b_w, lv_sb[:, 3:4],
                        start=True, stop=True,
                    )
                    eexp = consts.tile([128, 2], F32, tag="eexp")
                    nc.scalar.activation(eexp, dots_ps, EXP)
                    nc.vector.tensor_sub(lamc, eexp[:, 0:1], eexp[:, 1:2])
                    nc.vector.tensor_scalar_add(lamc, lamc, LAMBDA_INIT)
                    nc.sync.dma_start(padbc, padbc128.ap())

                # ============ phase 2 ============
                ATILE = 1024  # psA tile width (2 PSUM banks)
                with (
                    tc.tile_pool(name="psA", bufs=2, space="PSUM") as psA,
                    tc.tile_pool(name="sb2", bufs=2) as sb2,
                    tc.tile_pool(name="obufs", bufs=4) as obufs,
                    tc.tile_pool(name="rbufs", bufs=4) as rbufs,
                ):
                    def stage_a(j):
                        """Scores -> mask -> exp into u bands, both branches."""
                        q_lo = 0 if j == 0 else 128 * (j - 1)
                        w = widths[j]
                        for br in range(2):
                            KT = (k1t, k2t)[br]
                            QT = (q1t, q2t)[br]
                            for t0 in range(0, w, ATILE):
                                tw = min(ATILE, w - t0)
                                s_ps = psA.tile([128, ATILE], F32, tag="s")
                                for n0 in range(0, tw, 512):
                                    nw = min(512, tw - n0)
                                    nc.tensor.matmul(
                                        s_ps[:, n0 : n0 + nw],
                                        KT[:, j * 128 : (j + 1) * 128],
                                        QT[:, q_lo + t0 + n0 : q_lo + t0 + n0 + nw],
                                        start=True,
                                        stop=True,
                                    )
                                if t0 == 0:
                                    # superdiag+diag triangular mask (-> u = 1)
                                    if j == 0:
                                        mask_eng.tensor_mul(
                                            s_ps[:, 0:128], s_ps[:, 0:128], trid_sb
                                        )
                                    else:
                                        mask_eng.tensor_mul(
                                            s_ps[:, 0:256], s_ps[:, 0:256], trisd_sb
                                        )
                                nc.scalar.activation(
                                    u_band[br][
                                        :, band_off[j] + t0 : band_off[j] + t0 + tw
                                    ],
                                    s_ps[:, :tw],
                                    EXP,
                                    scale=SCALE,
                                )

                    # ---- V projection interleaved with stage A j=0..4 ----
                    with tc.tile_pool(name="ppV", bufs=1, space="PSUM") as ppV:
                        vps = ppV.tile([128, T], F32, tag="vproj")
                        vtb = obufs.tile([128, T], BF16, tag="vtb", bufs=1)
                        for ct in range(nC):
                            xt = xs.tile([128, T], BF16, tag="xt")
                            nc.sync.dma_start(xt, vT.ap()[ct * 128 : (ct + 1) * 128, :])
                            if ct < 5 and not phase1_only:
                                stage_a(ct)
                            for n0 in range(0, T, 512):
                                nc.tensor.matmul(
                                    vps[:, n0 : n0 + 512],
                                    wv_w(ct),
                                    xt[:, n0 : n0 + 512],
                                    start=(ct == 0),
                                    stop=(ct == nC - 1),
                                )
                        for n0 in range(0, T, 1024):
                            nc.vector.tensor_copy(
                                vtb[:, n0 : n0 + 1024], vps[:, n0 : n0 + 1024]
                            )

                    # V natural blocks + per-block column sums + suffix sums
                    with tc.tile_pool(name="ppT", bufs=2, space="PSUM") as ppT:
                        for j in range(nT):
                            vtr = ppT.tile([128, 128], BF16, tag="m")
                            nc.tensor.transpose(
                                vtr, vtb[:, j * 128 : (j + 1) * 128], idb_w
                            )
                            nc.vector.tensor_copy(vsb[:, j, :], vtr)
                    nc.vector.tensor_reduce(
                        vcols,
                        vtb.rearrange("p (j q) -> p j q", j=nT),
                        mybir.AxisListType.X,
                        mybir.AluOpType.add,
                    )
                    nc.vector.memset(sufv[:, nT - 1 : nT], 0.0)
                    nc.vector.memset(sufv[:, nT - 2 : nT - 1], 0.0)
                    for i in range(nT - 3, -1, -1):
                        nc.vector.tensor_add(
                            sufv[:, i : i + 1], sufv[:, i + 1 : i + 2],
                            vcols[:, i + 2 : i + 3],
                        )
                    nc.vector.tensor_add(tvn, sufv[:, 0:1], vcols[:, 0:1])
                    nc.vector.tensor_add(tvn, tvn, vcols[:, 1:2])
                    nc.vector.tensor_scalar_mul(tvn, tvn, 1.0 / T)
                    # pad rows see uniform attention in both branches:
                    # out = (1 - lambda) * mean(V)
                    tvl = consts.tile([128, 1], F32, tag="tvl")
                    nc.vector.tensor_scalar(
                        tvl, tvn, lamc, None, mybir.AluOpType.mult
                    )
                    nc.vector.tensor_sub(tvn, tvn, tvl)

                    # ---- stage B + remaining stage A groups ----
                    pending = []

                    def emit_combine(q0, br_res, last=False):
                        # oT = O1*R1 - lambda*O2*R2; pad rows -> uniform value.
                        # The final chunk's combine skips the Pool handoff: PE
                        # is done, so the shortest serial chain wins.
                        ceng = nc.vector if last else comb_eng
                        (ou1sb, r1bc), (ou2sb, r2bc) = br_res
                        c1 = sb2.tile([128, NQ], F32, tag="c1")
                        ceng.tensor_mul(c1, ou1sb, r1bc)
                        c2 = sb2.tile([128, NQ], F32, tag="c2")
                        nc.vector.scalar_tensor_tensor(
                            c2, ou2sb, lamc, r2bc,
                            mybir.AluOpType.mult, mybir.AluOpType.mult,
                        )
                        oT = sb2.tile([128, NQ], F32, tag="oT")
                        ceng.tensor_sub(oT, c1, c2)
                        oT2 = sb2.tile([128, NQ], F32, tag="oT2")
                        nc.vector.scalar_tensor_tensor(
                            oT2, padbc[:, q0 : q0 + NQ], tvn, oT,
                            mybir.AluOpType.mult, mybir.AluOpType.add,
                        )
                        nc.sync.dma_start(out.ap()[:, q0 : q0 + NQ], oT2)

                    def b_window(c, j):
                        q0 = c * NQ
                        q_lo = 0 if j == 0 else 128 * (j - 1)
                        oo = max(0, q_lo - q0)
                        wo = max(0, q0 - q_lo)
                        return q0, oo, wo, NQ - oo

                    class BChunk:
                        """One query chunk of stage B, emitted in j-slices so
                        the PE stream can alternate with stage-A groups."""

                        def __init__(self, c, psO, psD):
                            self.c = c
                            self.psD = psD
                            self.q0 = c * NQ
                            self.jmax = min((self.q0 + NQ) // 128, nT - 1)
                            if pending:
                                emit_combine(*pending.pop(0))
                            self.ou_h = [
                                psO.tile([128, NQ], F32, tag="ou", name=f"ou{br}")
                                for br in range(2)
                            ]
                            self.d2ps = psD.tile([128, NQ], F32, tag="dd")
                            # masked-count row seeds the denominator chain
                            for br in range(2):
                                nc.tensor.matmul(
                                    self.d2ps[32 * br : 32 * br + 1, :],
                                    onesb_w[0:1, :],
                                    sufb_sb[:, self.q0 : self.q0 + NQ],
                                    start=True,
                                    stop=False,
                                    skip_group_check=True,
                                )

                        def dsum_slice(self, js):
                            for br in range(2):
                                dps = self.d2ps[32 * br : 32 * br + 1, :]
                                for j in js:
                                    _, oo, wo, w = b_window(self.c, j)
                                    nc.tensor.matmul(
                                        dps[:, oo : oo + w],
                                        onesb_w,
                                        u_band[br][
                                            :, band_off[j] + wo : band_off[j] + wo + w
                                        ],
                                        start=False,
                                        stop=(j == self.jmax),
                                        skip_group_check=True,
                                    )

                        def uv_slice(self, js):
                            for j in js:
                                _, oo, wo, w = b_window(self.c, j)
                                for br in range(2):
                                    nc.tensor.matmul(
                                        self.ou_h[br][:, oo : oo + w],
                                        vsb[:, j, :],
                                        u_band[br][
                                            :, band_off[j] + wo : band_off[j] + wo + w
                                        ],
                                        start=(j == 0),
                                        stop=(j == self.jmax),
                                        skip_group_check=True,
                                    )

                        def slice(self, js, last=False):
                            # denominators first: on the last slice their
                            # reciprocal/broadcast chain then overlaps the
                            # remaining U@V matmuls
                            self.dsum_slice(js)
                            if last:
                                self.epi_d()
                            self.uv_slice(js)

                        def epi_d(self):
                            q0 = self.q0
                            self.rbcs = []
                            for br in range(2):
                                dps = self.d2ps[32 * br : 32 * br + 1, :]
                                rrow = sb2.tile([1, NQ], BF16, tag="rrow")
                                with nc.allow_low_precision(
                                    reason="bf16 recip row feeds a bf16 "
                                    "broadcast matmul; 0.4% is within budget"
                                ):
                                    nc.vector.reciprocal(rrow, dps)
                                    # zero padded cols (pad handled in combine)
                                    nc.vector.tensor_mul(
                                        rrow, rrow, npad_sb[:, q0 : q0 + NQ]
                                    )
                                rb_ps = self.psD.tile([128, NQ], F32, tag="rb")
                                nc.tensor.matmul(
                                    rb_ps, onesr_bw, rrow, start=True, stop=True
                                )
                                rbc = rbufs.tile([128, NQ], F32, tag="rbc")
                                if br == 0:
                                    nc.vector.tensor_copy(rbc, rb_ps)
                                else:
                                    nc.scalar.copy(rbc, rb_ps)
                                self.rbcs.append(rbc)

                        def epilogue(self):
                            # drain O_u (adding skipped-region suffix V sums),
                            # defer the combine
                            q0 = self.q0
                            br_res = []
                            for br in range(2):
                                ousb = obufs.tile([128, NQ], F32, tag="ousb")
                                nb = NQ // 128
                                nc.vector.tensor_tensor(
                                    ousb.rearrange("p (b q) -> p b q", b=nb),
                                    self.ou_h[br].rearrange("p (b q) -> p b q", b=nb),
                                    sufv[:, q0 // 128 : q0 // 128 + nb]
                                    .unsqueeze(2)
                                    .broadcast_to([128, nb, 128]),
                                    mybir.AluOpType.add,
                                )
                                br_res.append((ousb, self.rbcs[br]))
                            pending.append((q0, br_res))

                    if not phase1_only:
                        with (
                            tc.tile_pool(name="psO", bufs=2, space="PSUM") as psO,
                            tc.tile_pool(name="psD", bufs=1, space="PSUM") as psD,
                        ):
                            # Fine-grained interleave: single A groups alternate
                            # with j-slices of the one active B chunk, so the
                            # in-order PE stream never sits long behind the ACT
                            # exp queue (psA is only 2 tiles deep).
                            B = lambda c: BChunk(c, psO, psD)
                            b = B(0)
                            stage_a(5)
                            stage_a(6)
                            b.slice(range(0, 3))
                            stage_a(7)
                            b.slice(range(3, 5), last=True)
                            b.epilogue()
                            stage_a(8)
                            b = B(1)
                            b.slice(range(0, 3))
                            stage_a(9)
                            b.slice(range(3, 6))
                            stage_a(10)
                            b.slice(range(6, 9), last=True)
                            b.epilogue()
                            stage_a(11)
                            b = B(2)
                            b.slice(range(0, 4))
                            stage_a(12)
                            b.slice(range(4, 8))
                            stage_a(13)
                            b.slice(range(8, 13), last=True)
                            b.epilogue()
                            stage_a(14)
                            stage_a(15)
                            b = B(3)
                            b.slice(range(0, 13))
                            b.slice(range(13, nT), last=True)
                            b.epilogue()
                            for args in pending:
                                emit_combine(*args, last=True)
                            pending = []

    nc.compile()
    return nc


def _host_constants(T, nT):
    kl = np.arange(128)[:, None]
    ql = np.arange(128)[None, :]
    trid = (kl <= ql + 1).astype(np.float32)
    tris = ((kl + 128) <= (ql + 1)).astype(np.float32)
    tiles = np.arange(T) // 128
    sufcnt = (128.0 * np.maximum(0, nT - 2 - tiles)).astype(np.float32)[None, :]
    return trid, tris, sufcnt


_NC_CACHE = {}


def make_in_maps(q, k, v, pad_mask, Wq, Wk, Wv, lq1, lk1, lq2, lk2):
    """Per-core input dicts (host-side sharding + layout marshaling)."""
    B, T, C = q.shape
    nT, nC = T // 128, C // 128
    bf16 = ml_dtypes.bfloat16
    trid, tris, sufcnt = _host_constants(T, nT)

    def blocked(W):
        # [C, D] -> [128, nC*D] with cols ct*D + d = W[ct*128 + p, d]
        D = W.shape[1]
        return (
            np.asarray(W)
            .reshape(nC, 128, D)
            .transpose(1, 0, 2)
            .reshape(128, nC * D)
        )

    cbK = np.ascontiguousarray(blocked(Wk)).astype(bf16)
    cbR = np.concatenate(
        [
            blocked(Wq),
            blocked(Wv),
            np.eye(128, dtype=np.float32),
            np.ones((128, 1), dtype=np.float32),
            np.ones((128, 128), dtype=np.float32),
        ],
        axis=1,
    ).astype(bf16)
    lvec = np.stack(
        [np.asarray(lq1), np.asarray(lk1), np.asarray(lq2), np.asarray(lk2)], axis=1
    ).astype(np.float32)
    lq1b = np.tile(np.asarray(lq1)[:, None], (1, 128))
    lq2b = np.tile(np.asarray(lq2)[:, None], (1, 128))
    cbf32 = np.concatenate([tris, trid, lvec, lq1b, lq2b], axis=1).astype(
        np.float32
    )
    sufcnt_bf = np.ascontiguousarray(sufcnt.astype(bf16))

    in_maps = []
    for b in range(B):
        padf = np.asarray(pad_mask[b], dtype=np.float32)
        pn2 = np.ascontiguousarray(
            np.concatenate([padf, 1.0 - padf])[None, :].astype(bf16)
        )
        padbc128 = np.ascontiguousarray(
            np.broadcast_to(padf[None, :], (128, T)).astype(bf16)
        )
        in_maps.append(
            dict(
                padbc128=padbc128,
                qT=np.ascontiguousarray(np.asarray(q[b]).T.astype(bf16)),
                kT=np.ascontiguousarray(np.asarray(k[b]).T.astype(bf16)),
                vT=np.ascontiguousarray(np.asarray(v[b]).T.astype(bf16)),
                cbK=cbK,
                cbR=cbR,
                cbf32=cbf32,
                pn2=pn2,
                sufcnt_bf=sufcnt_bf,
            )
        )
    return in_maps


def kernel(q, k, v, pad_mask, Wq, Wk, Wv, lq1, lk1, lq2, lk2):
    B, T, C = q.shape
    assert B == N_CORES
    key = (T, C)
    if key not in _NC_CACHE:
        _NC_CACHE[key] = build_nc(T=T, C=C)
    nc = _NC_CACHE[key]
    in_maps = make_in_maps(q, k, v, pad_mask, Wq, Wk, Wv, lq1, lk1, lq2, lk2)
    res = run_bass_kernel_spmd(nc, in_maps, core_ids=list(range(N_CORES)))
    return np.stack(
        [np.ascontiguousarray(r["out"].T) for r in res.results], axis=0
    )


# revision 23
# speedup vs baseline: 10.4675x; 1.0205x over previous
"""DiffHead (differential attention head) Trainium2 Bass kernel.

Strategy (hardcoded for B=8, T=2048, C=1024, HS=128, 8 cores):
  - Data-parallel over batch: one batch element per NeuronCore.
  - Host side only reshapes/shards: per-core q/k/v slices are passed
    transposed ([C, T]) so the projection matmuls contract over C on the
    partition axis. All FLOPs run on device.
  - Scores are computed transposed (S^T[k, q]); masked fills of 1e-9
    scale to exactly 1.0f after exp, so the fully-masked region beyond
    the diagonal/superdiagonal blocks is never computed: its
    contributions are closed-form (suffix sums of V rows + a masked
    count folded into the denominator matmul chain).
  - Phase 2 is split into stage A (scores -> mask -> exp into an SBUF
    "u band" per branch) and stage B (U@V + denominator matmuls +
    combine), manually interleaved so the PE never waits long on the
    ACT exp stream. The V projection itself interleaves with the first
    stage-A group to hide its DMA pacing.
  - All small constants + weights ship in packed blobs and inputs load
    one DMA per 128-row tile, keeping HWDGE descriptor generation off
    the critical path.
  - Row->all-partition broadcasts (reciprocal rows, pad row, lambda)
    and part of the combine run on the otherwise idle GpSimd engine.
"""

import numpy as np
import ml_dtypes

try:
    import concourse.bacc as bacc
except ImportError:  # pragma: no cover
    import sys

    sys.path.insert(0, "/opt/trn_rl_repo")
    import concourse.bacc as bacc

import concourse.mybir as mybir
import concourse.tile as tile
from concourse.bass_utils import run_bass_kernel_spmd

F32 = mybir.dt.float32
F32R = mybir.dt.float32r
BF16 = mybir.dt.bfloat16
EXP = mybir.ActivationFunctionType.Exp

HS = 128
LAMBDA_INIT = 0.8
N_CORES = 8

# feature flags (fallbacks if an op turns out unsupported)
MASK_ON_POOL = False
COMBINE_ON_POOL = False


def _r(ap):
    """View an f32 AP as float32r so the PE runs at full rate."""
    return ap.bitcast(F32R)


def _band_widths(T, nT):
    """Score-band column width per k-tile j: queries q >= 128*(j-1)."""
    return [T - (0 if j == 0 else 128 * (j - 1)) for j in range(nT)]


def build_nc(T=2048, C=1024, NQ=512, repeat=1, phase1_only=False):
    """Build the per-core Bass program. Same NEFF on all 8 cores (SPMD).

    repeat > 1 wraps the body in a hardware loop (for wall-clock slope
    timing); results are identical since the body is idempotent.
    """
    import contextlib

    nT = T // 128
    nC = C // 128
    NQ = min(NQ, T)
    SCALE = float(HS) ** -0.5

    widths = _band_widths(T, nT)
    band_off = [0] * nT
    for j in range(1, nT):
        band_off[j] = band_off[j - 1] + widths[j - 1]
    band_cols = band_off[-1] + widths[-1]

    # K weights ship alone (needed first); the rest packs into cbR:
    # [wq | wv | idb | onesb]
    KCOLS = nC * 2 * HS
    WQ0 = 0
    WV0 = WQ0 + nC * 2 * HS
    IDB0 = WV0 + nC * HS
    ONE0 = IDB0 + 128
    ONESR0 = ONE0 + 1
    RCOLS = ONESR0 + 128
    # packed f32 blob offsets: [tris | trid | lvec | tiled lq1 | tiled lq2]
    TRIS0, TRID0, LV0 = 0, 128, 256
    LQ1B0 = LV0 + 4
    LQ2B0 = LQ1B0 + 128
    F32COLS = LQ2B0 + 128

    nc = bacc.Bacc("TRN2", target_bir_lowering=False, num_devices=N_CORES)

    qT = nc.dram_tensor("qT", [C, T], BF16, kind="ExternalInput")
    kT = nc.dram_tensor("kT", [C, T], BF16, kind="ExternalInput")
    vT = nc.dram_tensor("vT", [C, T], BF16, kind="ExternalInput")
    cbK = nc.dram_tensor("cbK", [128, KCOLS], BF16, kind="ExternalInput")
    cbR = nc.dram_tensor("cbR", [128, RCOLS], BF16, kind="ExternalInput")
    cbf32 = nc.dram_tensor("cbf32", [128, F32COLS], F32, kind="ExternalInput")
    pn2 = nc.dram_tensor("pn2", [1, 2 * T], BF16, kind="ExternalInput")
    padbc128 = nc.dram_tensor("padbc128", [128, T], BF16, kind="ExternalInput")
    sufcnt_bf = nc.dram_tensor("sufcnt_bf", [1, T], BF16, kind="ExternalInput")
    # output stays transposed ([dv, T]); the host un-transposes.
    out = nc.dram_tensor("out", [HS, T], F32, kind="ExternalOutput")

    with tile.TileContext(nc) as tc:
        rep_cm = tc.For_i(0, repeat, 1) if repeat > 1 else contextlib.nullcontext()
        with (
            rep_cm,
            tc.tile_pool(name="consts", bufs=1) as consts,
            tc.tile_pool(name="persist", bufs=1) as persist,
        ):
            # ---- packed constants (K weights first: needed immediately) ----
            cbk_sb = consts.tile([128, KCOLS], BF16, tag="cbK")
            nc.sync.dma_start(cbk_sb, cbK.ap())
            cbr_sb = consts.tile([128, RCOLS], BF16, tag="cbR")
            cf = consts.tile([128, F32COLS], F32, tag="cbf32")
            pn_sb = consts.tile([1, 2 * T], BF16, tag="pn2")
            sufb_sb = consts.tile([1, T], BF16, tag="sufb")

            def wk_w(ct, h):
                return cbk_sb[:, ct * 2 * HS + h * HS : ct * 2 * HS + (h + 1) * HS]

            def wq_w(ct, h):
                return cbr_sb[:, WQ0 + ct * 2 * HS + h * HS : WQ0 + ct * 2 * HS + (h + 1) * HS]

            def wv_w(ct):
                return cbr_sb[:, WV0 + ct * HS : WV0 + (ct + 1) * HS]

            idb_w = cbr_sb[:, IDB0 : IDB0 + 128]
            onesb_w = cbr_sb[:, ONE0 : ONE0 + 1]
            onesr_bw = cbr_sb[0:1, ONESR0 : ONESR0 + 128]
            lq1b_w = cf[:, LQ1B0 : LQ1B0 + 128]
            lq2b_w = cf[:, LQ2B0 : LQ2B0 + 128]
            trisd_sb = cf[:, TRIS0 : TRIS0 + 256]
            trid_sb = cf[:, TRID0 : TRID0 + 128]
            lv_sb = cf[:, LV0 : LV0 + 4]
            pad_sb = pn_sb[:, 0:T]
            npad_sb = pn_sb[:, T : 2 * T]

            # ---- persistent intermediates ----
            q1t = persist.tile([128, T], F32R, tag="q1t")
            q2t = persist.tile([128, T], F32R, tag="q2t")
            k1t = persist.tile([128, T], F32R, tag="k1t")
            k2t = persist.tile([128, T], F32R, tag="k2t")
            vsb = persist.tile([128, nT, 128], BF16, tag="vsb")  # V natural
            padbc = persist.tile([128, T], BF16, tag="padbc")
            vcols = persist.tile([128, nT], F32, tag="vcols")
            sufv = persist.tile([128, nT], F32, tag="sufv")
            tvn = persist.tile([128, 1], F32, tag="tvn")  # (1-lambda)*mean(V)
            lamc = persist.tile([128, 1], F32, tag="lamc")
            u_band = [
                persist.tile([128, band_cols], BF16, tag="u1", name="u1"),
                persist.tile([128, band_cols], BF16, tag="u2", name="u2"),
            ]

            mask_eng = nc.gpsimd if MASK_ON_POOL else nc.vector
            comb_eng = nc.gpsimd if COMBINE_ON_POOL else nc.vector

            with tc.tile_pool(name="xs", bufs=6) as xs:
                # ============ K and Q projections (full-PSUM scope) ============
                with tc.tile_pool(name="ppK", bufs=2, space="PSUM") as ppK:
                    first = True
                    for xdram, w_of, outs in (
                        (kT, wk_w, (k1t, k2t)),
                        (qT, wq_w, (q1t, q2t)),
                    ):
                        ps = [
                            ppK.tile([128, T], F32, tag="proj", name=f"ps{h}")
                            for h in range(2)
                        ]
                        for ct in range(nC):
                            xt = xs.tile([128, T], BF16, tag="xt")
                            nc.sync.dma_start(
                                xt, xdram.ap()[ct * 128 : (ct + 1) * 128, :]
                            )
                            if first:
                                # remaining consts ride behind the first tile
                                nc.sync.dma_start(cbr_sb, cbR.ap())
                                nc.sync.dma_start(cf, cbf32.ap())
                                nc.sync.dma_start(pn_sb, pn2.ap())
                                nc.sync.dma_start(sufb_sb, sufcnt_bf.ap())
                                first = False
                            for h in range(2):
                                for n0 in range(0, T, 512):
                                    nc.tensor.matmul(
                                        ps[h][:, n0 : n0 + 512],
                                        w_of(ct, h),
                                        xt[:, n0 : n0 + 512],
                                        start=(ct == 0),
                                        stop=(ct == nC - 1),
                                    )
                        for h in range(2):
                            for ni, n0 in enumerate(range(0, T, 1024)):
                                if (h + ni) % 2 == 0:
                                    nc.scalar.copy(
                                        outs[h][:, n0 : n0 + 1024],
                                        ps[h][:, n0 : n0 + 1024],
                                    )
                                else:
                                    nc.vector.tensor_copy(
                                        outs[h][:, n0 : n0 + 1024],
                                        ps[h][:, n0 : n0 + 1024],
                                    )

                # ============ lambda (broadcast via host-tiled weights) ============
                with tc.tile_pool(name="ppL", bufs=1, space="PSUM") as ppL:
                    # dots_bc[p, i] = sum_d lqi[d] * lki[d] for every p: the
                    # weights are lq1/lq2 replicated across columns on host
                    dots_ps = ppL.tile([128, 2], F32, tag="t")
                    nc.tensor.matmul(
                        dots_ps[:, 0:1], lq1b_w, lv_sb[:, 1:2],
                        start=True, stop=True,
                    )
                    nc.tensor.matmul(
                        dots_ps[:, 1:2], lq2b_w, lv_sb[:, 3:4],
                        start=True, stop=True,
                    )
                    eexp = consts.tile([128, 2], F32, tag="eexp")
                    nc.scalar.activation(eexp, dots_ps, EXP)
                    nc.vector.tensor_sub(lamc, eexp[:, 0:1], eexp[:, 1:2])
                    nc.vector.tensor_scalar_add(lamc, lamc, LAMBDA_INIT)
                    nc.sync.dma_start(padbc, padbc128.ap())

                # ============ phase 2 ============
                ATILE = 1024  # psA tile width (2 PSUM banks)
                with (
                    tc.tile_pool(name="psA", bufs=2, space="PSUM") as psA,
                    tc.tile_pool(name="sb2", bufs=2) as sb2,
                    tc.tile_pool(name="obufs", bufs=4) as obufs,
                    tc.tile_pool(name="rbufs", bufs=4) as rbufs,
                ):
                    def stage_a(j):
                        """Scores -> mask -> exp into u bands, both branches."""
                        q_lo = 0 if j == 0 else 128 * (j - 1)
                        w = widths[j]
                        for br in range(2):
                            KT = (k1t, k2t)[br]
                            QT = (q1t, q2t)[br]
                            for t0 in range(0, w, ATILE):
                                tw = min(ATILE, w - t0)
                                s_ps = psA.tile([128, ATILE], F32, tag="s")
                                for n0 in range(0, tw, 512):
                                    nw = min(512, tw - n0)
                                    nc.tensor.matmul(
                                        s_ps[:, n0 : n0 + nw],
                                        KT[:, j * 128 : (j + 1) * 128],
                                        QT[:, q_lo + t0 + n0 : q_lo + t0 + n0 + nw],
                                        start=True,
                                        stop=True,
                                    )
                                if t0 == 0:
                                    # superdiag+diag triangular mask (-> u = 1)
                                    if j == 0:
                                        mask_eng.tensor_mul(
                                            s_ps[:, 0:128], s_ps[:, 0:128], trid_sb
                                        )
                                    else:
                                        mask_eng.tensor_mul(
                                            s_ps[:, 0:256], s_ps[:, 0:256], trisd_sb
                                        )
                                nc.scalar.activation(
                                    u_band[br][
                                        :, band_off[j] + t0 : band_off[j] + t0 + tw
                                    ],
                                    s_ps[:, :tw],
                                    EXP,
                                    scale=SCALE,
                                )

                    # ---- V projection interleaved with stage A j=0..4 ----
                    with tc.tile_pool(name="ppV", bufs=1, space="PSUM") as ppV:
                        vps = ppV.tile([128, T], F32, tag="vproj")
                        vtb = obufs.tile([128, T], BF16, tag="vtb", bufs=1)
                        for ct in range(nC):
                            xt = xs.tile([128, T], BF16, tag="xt")
                            nc.sync.dma_start(xt, vT.ap()[ct * 128 : (ct + 1) * 128, :])
                            for n0 in range(0, T, 512):
                                nc.tensor.matmul(
                                    vps[:, n0 : n0 + 512],
                                    wv_w(ct),
                                    xt[:, n0 : n0 + 512],
                                    start=(ct == 0),
                                    stop=(ct == nC - 1),
                                )
                            if ct < 5 and not phase1_only:
                                stage_a(ct)
                        for n0 in range(0, T, 1024):
                            nc.vector.tensor_copy(
                                vtb[:, n0 : n0 + 1024], vps[:, n0 : n0 + 1024]
                            )

                    # V natural blocks + per-block column sums + suffix sums
                    with tc.tile_pool(name="ppT", bufs=2, space="PSUM") as ppT:
                        for j in range(nT):
                            vtr = ppT.tile([128, 128], BF16, tag="m")
                            nc.tensor.transpose(
                                vtr, vtb[:, j * 128 : (j + 1) * 128], idb_w
                            )
                            nc.vector.tensor_copy(vsb[:, j, :], vtr)
                    nc.vector.tensor_reduce(
                        vcols,
                        vtb.rearrange("p (j q) -> p j q", j=nT),
                        mybir.AxisListType.X,
                        mybir.AluOpType.add,
                    )
                    nc.vector.memset(sufv[:, nT - 1 : nT], 0.0)
                    nc.vector.memset(sufv[:, nT - 2 : nT - 1], 0.0)
                    for i in range(nT - 3, -1, -1):
                        nc.vector.tensor_add(
                            sufv[:, i : i + 1], sufv[:, i + 1 : i + 2],
                            vcols[:, i + 2 : i + 3],
                        )
                    nc.vector.tensor_add(tvn, sufv[:, 0:1], vcols[:, 0:1])
                    nc.vector.tensor_add(tvn, tvn, vcols[:, 1:2])
                    nc.vector.tensor_scalar_mul(tvn, tvn, 1.0 / T)
                    # pad rows see uniform attention in both branches:
                    # out = (1 - lambda) * mean(V)
                    tvl = consts.tile([128, 1], F32, tag="tvl")
                    nc.vector.tensor_scalar(
                        tvl, tvn, lamc, None, mybir.AluOpType.mult
                    )
                    nc.vector.tensor_sub(tvn, tvn, tvl)

                    # ---- stage B + remaining stage A groups ----
                    pending = []

                    def emit_combine(q0, br_res, last=False):
                        # oT = O1*R1 - lambda*O2*R2; pad rows -> uniform value.
                        # The final chunk's combine skips the Pool handoff: PE
                        # is done, so the shortest serial chain wins.
                        ceng = nc.vector if last else comb_eng
                        (ou1sb, r1bc), (ou2sb, r2bc) = br_res
                        c1 = sb2.tile([128, NQ], F32, tag="c1")
                        ceng.tensor_mul(c1, ou1sb, r1bc)
                        c2 = sb2.tile([128, NQ], F32, tag="c2")
                        nc.vector.scalar_tensor_tensor(
                            c2, ou2sb, lamc, r2bc,
                            mybir.AluOpType.mult, mybir.AluOpType.mult,
                        )
                        oT = sb2.tile([128, NQ], F32, tag="oT")
                        ceng.tensor_sub(oT, c1, c2)
                        oT2 = sb2.tile([128, NQ], F32, tag="oT2")
                        nc.vector.scalar_tensor_tensor(
                            oT2, padbc[:, q0 : q0 + NQ], tvn, oT,
                            mybir.AluOpType.mult, mybir.AluOpType.add,
                        )
                        nc.sync.dma_start(out.ap()[:, q0 : q0 + NQ], oT2)

                    def b_window(c, j):
                        q0 = c * NQ
                        q_lo = 0 if j == 0 else 128 * (j - 1)
                        oo = max(0, q_lo - q0)
                        wo = max(0, q0 - q_lo)
                        return q0, oo, wo, NQ - oo

                    class BChunk:
                        """One query chunk of stage B, emitted in j-slices so
                        the PE stream can alternate with stage-A groups."""

                        def __init__(self, c, psO, psD):
                            self.c = c
                            self.psD = psD
                            self.q0 = c * NQ
                            self.jmax = min((self.q0 + NQ) // 128, nT - 1)
                            if pending:
                                emit_combine(*pending.pop(0))
                            self.ou_h = [
                                psO.tile([128, NQ], F32, tag="ou", name=f"ou{br}")
                                for br in range(2)
                            ]
                            self.d2ps = psD.tile([128, NQ], F32, tag="dd")
                            # masked-count row seeds the denominator chain
                            for br in range(2):
                                nc.tensor.matmul(
                                    self.d2ps[32 * br : 32 * br + 1, :],
                                    onesb_w[0:1, :],
                                    sufb_sb[:, self.q0 : self.q0 + NQ],
                                    start=True,
                                    stop=False,
                                    skip_group_check=True,
                                )

                        def dsum_slice(self, js):
                            for br in range(2):
                                dps = self.d2ps[32 * br : 32 * br + 1, :]
                                for j in js:
                                    _, oo, wo, w = b_window(self.c, j)
                                    nc.tensor.matmul(
                                        dps[:, oo : oo + w],
                                        onesb_w,
                                        u_band[br][
                                            :, band_off[j] + wo : band_off[j] + wo + w
                                        ],
                                        start=False,
                                        stop=(j == self.jmax),
                                        skip_group_check=True,
                                    )

                        def uv_slice(self, js):
                            for j in js:
                                _, oo, wo, w = b_window(self.c, j)
                                for br in range(2):
                                    nc.tensor.matmul(
                                        self.ou_h[br][:, oo : oo + w],
                                        vsb[:, j, :],
                                        u_band[br][
                                            :, band_off[j] + wo : band_off[j] + wo + w
                                        ],
                                        start=(j == 0),
                                        stop=(j == self.jmax),
                                        skip_group_check=True,
                                    )

                        def slice(self, js, last=False):
                            # denominators first: on the last slice their
                            # reciprocal/broadcast chain then overlaps the
                            # remaining U@V matmuls
                            self.dsum_slice(js)
                            if last:
                                self.epi_d()
                            self.uv_slice(js)

                        def epi_d(self):
                            q0 = self.q0
                            self.rbcs = []
                            for br in range(2):
                                dps = self.d2ps[32 * br : 32 * br + 1, :]
                                rrow = sb2.tile([1, NQ], BF16, tag="rrow")
                                with nc.allow_low_precision(
                                    reason="bf16 recip row feeds a bf16 "
                                    "broadcast matmul; 0.4% is within budget"
                                ):
                                    nc.vector.reciprocal(rrow, dps)
                                    # zero padded cols (pad handled in combine)
                                    nc.vector.tensor_mul(
                                        rrow, rrow, npad_sb[:, q0 : q0 + NQ]
                                    )
                                rb_ps = self.psD.tile([128, NQ], F32, tag="rb")
                                nc.tensor.matmul(
                                    rb_ps, onesr_bw, rrow, start=True, stop=True
                                )
                                rbc = rbufs.tile([128, NQ], F32, tag="rbc")
                                if br == 0:
                                    nc.vector.tensor_copy(rbc, rb_ps)
                                else:
                                    nc.scalar.copy(rbc, rb_ps)
                                self.rbcs.append(rbc)

                        def epilogue(self):
                            # drain O_u (adding skipped-region suffix V sums),
                            # defer the combine
                            q0 = self.q0
                            br_res = []
                            for br in range(2):
                                ousb = obufs.tile([128, NQ], F32, tag="ousb")
                                nb = NQ // 128
                                nc.vector.tensor_tensor(
                                    ousb.rearrange("p (b q) -> p b q", b=nb),
                                    self.ou_h[br].rearrange("p (b q) -> p b q", b=nb),
                                    sufv[:, q0 // 128 : q0 // 128 + nb]
                                    .unsqueeze(2)
                                    .broadcast_to([128, nb, 128]),
                                    mybir.AluOpType.add,
                                )
                                br_res.append((ousb, self.rbcs[br]))
                            pending.append((q0, br_res))

                    if not phase1_only:
                        with (
                            tc.tile_pool(name="psO", bufs=2, space="PSUM") as psO,
                            tc.tile_pool(name="psD", bufs=1, space="PSUM") as psD,
                        ):
                            # Fine-grained interleave: single A groups alternate
                            # with j-slices of the one active B chunk, so the
                            # in-order PE stream never sits long behind the ACT
                            # exp queue (psA is only 2 tiles deep).
                            B = lambda c: BChunk(c, psO, psD)
                            b = B(0)
                            stage_a(5)
                            stage_a(6)
                            b.slice(range(0, 3))
                            stage_a(7)
                            b.slice(range(3, 5), last=True)
                            b.epilogue()
                            stage_a(8)
                            b = B(1)
                            b.slice(range(0, 3))
                            stage_a(9)
                            b.slice(range(3, 6))
                            stage_a(10)
                            b.slice(range(6, 9), last=True)
                            b.epilogue()
                            stage_a(11)
                            b = B(2)
                            b.slice(range(0, 4))
                            stage_a(12)
                            b.slice(range(4, 8))
                            stage_a(13)
                            b.slice(range(8, 13), last=True)
                            b.epilogue()
                            stage_a(14)
                            stage_a(15)
                            b = B(3)
                            b.slice(range(0, 13))
                            b.slice(range(13, nT), last=True)
                            b.epilogue()
                            for args in pending:
                                emit_combine(*args, last=True)
                            pending = []

    nc.compile()
    return nc


def _host_constants(T, nT):
    kl = np.arange(128)[:, None]
    ql = np.arange(128)[None, :]
    trid = (kl <= ql + 1).astype(np.float32)
    tris = ((kl + 128) <= (ql + 1)).astype(np.float32)
    tiles = np.arange(T) // 128
    sufcnt = (128.0 * np.maximum(0, nT - 2 - tiles)).astype(np.float32)[None, :]
    return trid, tris, sufcnt


_NC_CACHE = {}


def make_in_maps(q, k, v, pad_mask, Wq, Wk, Wv, lq1, lk1, lq2, lk2):
    """Per-core input dicts (host-side sharding + layout marshaling)."""
    B, T, C = q.shape
    nT, nC = T // 128, C // 128
    bf16 = ml_dtypes.bfloat16
    trid, tris, sufcnt = _host_constants(T, nT)

    def blocked(W):
        # [C, D] -> [128, nC*D] with cols ct*D + d = W[ct*128 + p, d]
        D = W.shape[1]
        return (
            np.asarray(W)
            .reshape(nC, 128, D)
            .transpose(1, 0, 2)
            .reshape(128, nC * D)
        )

    cbK = np.ascontiguousarray(blocked(Wk)).astype(bf16)
    cbR = np.concatenate(
        [
            blocked(Wq),
            blocked(Wv),
            np.eye(128, dtype=np.float32),
            np.ones((128, 1), dtype=np.float32),
            np.ones((128, 128), dtype=np.float32),
        ],
        axis=1,
    ).astype(bf16)
    lvec = np.stack(
        [np.asarray(lq1), np.asarray(lk1), np.asarray(lq2), np.asarray(lk2)], axis=1
    ).astype(np.float32)
    lq1b = np.tile(np.asarray(lq1)[:, None], (1, 128))
    lq2b = np.tile(np.asarray(lq2)[:, None], (1, 128))
    cbf32 = np.concatenate([tris, trid, lvec, lq1b, lq2b], axis=1).astype(
        np.float32
    )
    sufcnt_bf = np.ascontiguousarray(sufcnt.astype(bf16))

    in_maps = []
    for b in range(B):
        padf = np.asarray(pad_mask[b], dtype=np.float32)
        pn2 = np.ascontiguousarray(
            np.concatenate([padf, 1.0 - padf])[None, :].astype(bf16)
        )
        padbc128 = np.ascontiguousarray(
            np.broadcast_to(padf[None, :], (128, T)).astype(bf16)
        )
        in_maps.append(
            dict(
                padbc128=padbc128,
                qT=np.ascontiguousarray(np.asarray(q[b]).T.astype(bf16)),
                kT=np.ascontiguousarray(np.asarray(k[b]).T.astype(bf16)),
                vT=np.ascontiguousarray(np.asarray(v[b]).T.astype(bf16)),
                cbK=cbK,
                cbR=cbR,
                cbf32=cbf32,
                pn2=pn2,
                sufcnt_bf=sufcnt_bf,
            )
        )
    return in_maps


def kernel(q, k, v, pad_mask, Wq, Wk, Wv, lq1, lk1, lq2, lk2):
    B, T, C = q.shape
    assert B == N_CORES
    key = (T, C)
    if key not in _NC_CACHE:
        _NC_CACHE[key] = build_nc(T=T, C=C)
    nc = _NC_CACHE[key]
    in_maps = make_in_maps(q, k, v, pad_mask, Wq, Wk, Wv, lq1, lk1, lq2, lk2)
    res = run_bass_kernel_spmd(nc, in_maps, core_ids=list(range(N_CORES)))
    return np.stack(
        [np.ascontiguousarray(r["out"].T) for r in res.results], axis=0
    )


# revision 26
# speedup vs baseline: 11.3611x; 1.0854x over previous
"""DiffHead (differential attention head) Trainium2 Bass kernel.

Strategy (hardcoded for B=8, T=2048, C=1024, HS=128, 8 cores):
  - Data-parallel over batch: one batch element per NeuronCore.
  - Host side only reshapes/shards: per-core q/k/v slices are passed
    transposed ([C, T]) so the projection matmuls contract over C on the
    partition axis. All FLOPs run on device.
  - Scores are computed transposed (S^T[k, q]); masked fills of 1e-9
    scale to exactly 1.0f after exp, so the fully-masked region beyond
    the diagonal/superdiagonal blocks is never computed: its
    contributions are closed-form (suffix sums of V rows + a masked
    count folded into the denominator matmul chain).
  - Phase 2 is split into stage A (scores -> mask -> exp into an SBUF
    "u band" per branch) and stage B (U@V + denominator matmuls +
    combine), manually interleaved so the PE never waits long on the
    ACT exp stream. The V projection itself interleaves with the first
    stage-A group to hide its DMA pacing.
  - All small constants + weights ship in packed blobs and inputs load
    one DMA per 128-row tile, keeping HWDGE descriptor generation off
    the critical path.
  - Row->all-partition broadcasts (reciprocal rows, pad row, lambda)
    and part of the combine run on the otherwise idle GpSimd engine.
"""

import numpy as np
import ml_dtypes

try:
    import concourse.bacc as bacc
except ImportError:  # pragma: no cover
    import sys

    sys.path.insert(0, "/opt/trn_rl_repo")
    import concourse.bacc as bacc

import concourse.mybir as mybir
import concourse.tile as tile
from concourse.bass_utils import run_bass_kernel_spmd

F32 = mybir.dt.float32
F32R = mybir.dt.float32r
BF16 = mybir.dt.bfloat16
EXP = mybir.ActivationFunctionType.Exp

HS = 128
LAMBDA_INIT = 0.8
N_CORES = 8

# feature flags (fallbacks if an op turns out unsupported)
MASK_ON_POOL = False
COMBINE_ON_POOL = False


def _r(ap):
    """View an f32 AP as float32r so the PE runs at full rate."""
    return ap.bitcast(F32R)


def _band_widths(T, nT):
    """Score-band column width per k-tile j: queries q >= 128*(j-1)."""
    return [T - (0 if j == 0 else 128 * (j - 1)) for j in range(nT)]


def build_nc(T=2048, C=1024, NQ=512, repeat=1, phase1_only=False):
    """Build the per-core Bass program. Same NEFF on all 8 cores (SPMD).

    repeat > 1 wraps the body in a hardware loop (for wall-clock slope
    timing); results are identical since the body is idempotent.
    """
    import contextlib

    nT = T // 128
    nC = C // 128
    NQ = min(NQ, T)
    SCALE = float(HS) ** -0.5

    widths = _band_widths(T, nT)
    band_off = [0] * nT
    for j in range(1, nT):
        band_off[j] = band_off[j - 1] + widths[j - 1]
    band_cols = band_off[-1] + widths[-1]

    # K weights ship alone (needed first); the rest packs into cbR:
    # [wq | wv | idb | onesb]
    KCOLS = nC * 2 * HS
    WQ0 = 0
    WV0 = WQ0 + nC * 2 * HS
    IDB0 = WV0 + nC * HS
    ONE0 = IDB0 + 128
    ONESR0 = ONE0 + 1
    TRISB0 = ONESR0 + 128
    TRIS1M0 = TRISB0 + 256
    BLK0 = TRIS1M0 + 256
    RCOLS = BLK0 + 512
    # packed f32 blob offsets: [tris | trid | lvec | tiled lq1 | tiled lq2]
    TRIS0, TRID0, LV0 = 0, 128, 256
    LQ1B0 = LV0 + 4
    LQ2B0 = LQ1B0 + 128
    F32COLS = LQ2B0 + 128

    nc = bacc.Bacc("TRN2", target_bir_lowering=False, num_devices=N_CORES)

    qT = nc.dram_tensor("qT", [C, T], BF16, kind="ExternalInput")
    kT = nc.dram_tensor("kT", [C, T], BF16, kind="ExternalInput")
    vT = nc.dram_tensor("vT", [C, T], BF16, kind="ExternalInput")
    cbK = nc.dram_tensor("cbK", [128, KCOLS], BF16, kind="ExternalInput")
    cbR = nc.dram_tensor("cbR", [128, RCOLS], BF16, kind="ExternalInput")
    cbf32 = nc.dram_tensor("cbf32", [128, F32COLS], F32, kind="ExternalInput")
    pn2 = nc.dram_tensor("pn2", [1, 2 * T], BF16, kind="ExternalInput")
    padbc128 = nc.dram_tensor("padbc128", [128, T], BF16, kind="ExternalInput")
    sufcnt_bf = nc.dram_tensor("sufcnt_bf", [1, T], BF16, kind="ExternalInput")
    # output stays transposed ([dv, T]); the host un-transposes.
    out = nc.dram_tensor("out", [HS, T], F32, kind="ExternalOutput")

    with tile.TileContext(nc) as tc:
        rep_cm = tc.For_i(0, repeat, 1) if repeat > 1 else contextlib.nullcontext()
        with (
            rep_cm,
            tc.tile_pool(name="consts", bufs=1) as consts,
            tc.tile_pool(name="persist", bufs=1) as persist,
        ):
            # ---- packed constants (K weights first: needed immediately) ----
            cbk_sb = consts.tile([128, KCOLS], BF16, tag="cbK")
            # first ct-tile's weights land first so the PE can start sooner
            nc.sync.dma_start(cbk_sb[:, 0 : 2 * HS], cbK.ap()[:, 0 : 2 * HS])
            nc.sync.dma_start(
                cbk_sb[:, 2 * HS : KCOLS], cbK.ap()[:, 2 * HS : KCOLS]
            )
            cbr_sb = consts.tile([128, RCOLS], BF16, tag="cbR")
            cf = consts.tile([128, F32COLS], F32, tag="cbf32")
            pn_sb = consts.tile([1, 2 * T], BF16, tag="pn2")
            sufb_sb = consts.tile([1, T], BF16, tag="sufb")

            def wk_w(ct, h):
                return cbk_sb[:, ct * 2 * HS + h * HS : ct * 2 * HS + (h + 1) * HS]

            def wq_w(ct, h):
                return cbr_sb[:, WQ0 + ct * 2 * HS + h * HS : WQ0 + ct * 2 * HS + (h + 1) * HS]

            def wv_w(ct):
                return cbr_sb[:, WV0 + ct * HS : WV0 + (ct + 1) * HS]

            idb_w = cbr_sb[:, IDB0 : IDB0 + 128]
            onesb_w = cbr_sb[:, ONE0 : ONE0 + 1]
            onesr_bw = cbr_sb[0:1, ONESR0 : ONESR0 + 128]
            trisdb = cbr_sb[:, TRISB0 : TRISB0 + 256]
            trisd1m = cbr_sb[:, TRIS1M0 : TRIS1M0 + 256]
            blkind_w = cbr_sb[0:4, BLK0 : BLK0 + 512]
            lq1b_w = cf[:, LQ1B0 : LQ1B0 + 128]
            lq2b_w = cf[:, LQ2B0 : LQ2B0 + 128]
            trisd_sb = cf[:, TRIS0 : TRIS0 + 256]
            trid_sb = cf[:, TRID0 : TRID0 + 128]
            lv_sb = cf[:, LV0 : LV0 + 4]
            pad_sb = pn_sb[:, 0:T]
            npad_sb = pn_sb[:, T : 2 * T]

            # ---- persistent intermediates ----
            q1t = persist.tile([128, T], F32R, tag="q1t")
            q2t = persist.tile([128, T], F32R, tag="q2t")
            k1t = persist.tile([128, T], F32R, tag="k1t")
            k2t = persist.tile([128, T], F32R, tag="k2t")
            vsb = persist.tile([128, nT, 128], BF16, tag="vsb")  # V natural
            padbc = persist.tile([128, T], BF16, tag="padbc")
            vcols = persist.tile([128, nT], F32, tag="vcols")
            sufv = persist.tile([128, nT], F32, tag="sufv")
            tvn = persist.tile([128, 1], F32, tag="tvn")  # (1-lambda)*mean(V)
            lamc = persist.tile([128, 1], F32, tag="lamc")
            sufvT = persist.tile([16, 128], BF16, tag="sufvT")
            u_band = [
                persist.tile([128, band_cols], BF16, tag="u1", name="u1"),
                persist.tile([128, band_cols], BF16, tag="u2", name="u2"),
            ]

            mask_eng = nc.gpsimd if MASK_ON_POOL else nc.vector
            comb_eng = nc.gpsimd if COMBINE_ON_POOL else nc.vector

            with tc.tile_pool(name="xs", bufs=6) as xs:
                # ============ K and Q projections (full-PSUM scope) ============
                with tc.tile_pool(name="ppK", bufs=2, space="PSUM") as ppK:
                    first = True
                    for xdram, w_of, outs in (
                        (kT, wk_w, (k1t, k2t)),
                        (qT, wq_w, (q1t, q2t)),
                    ):
                        ps = [
                            ppK.tile([128, T], F32, tag="proj", name=f"ps{h}")
                            for h in range(2)
                        ]
                        for ct in range(nC):
                            xt = xs.tile([128, T], BF16, tag="xt")
                            nc.sync.dma_start(
                                xt, xdram.ap()[ct * 128 : (ct + 1) * 128, :]
                            )
                            if first and ct == 2:
                                # remaining consts ride behind the third tile
                                # (cbR is only needed from the Q projections)
                                nc.sync.dma_start(cbr_sb, cbR.ap())
                                nc.sync.dma_start(cf, cbf32.ap())
                                nc.sync.dma_start(pn_sb, pn2.ap())
                                nc.sync.dma_start(sufb_sb, sufcnt_bf.ap())
                                first = False
                            for h in range(2):
                                for n0 in range(0, T, 512):
                                    nc.tensor.matmul(
                                        ps[h][:, n0 : n0 + 512],
                                        w_of(ct, h),
                                        xt[:, n0 : n0 + 512],
                                        start=(ct == 0),
                                        stop=(ct == nC - 1),
                                    )
                                if ct == nC - 1:
                                    # drain each half right behind its stop so
                                    # the next tensor's PSUM frees sooner
                                    for ni, n0 in enumerate(range(0, T, 1024)):
                                        if (h + ni) % 2 == 0:
                                            nc.scalar.copy(
                                                outs[h][:, n0 : n0 + 1024],
                                                ps[h][:, n0 : n0 + 1024],
                                            )
                                        else:
                                            nc.vector.tensor_copy(
                                                outs[h][:, n0 : n0 + 1024],
                                                ps[h][:, n0 : n0 + 1024],
                                            )

                # ============ lambda (broadcast via host-tiled weights) ============
                with tc.tile_pool(name="ppL", bufs=1, space="PSUM") as ppL:
                    # dots_bc[p, i] = sum_d lqi[d] * lki[d] for every p: the
                    # weights are lq1/lq2 replicated across columns on host
                    dots_ps = ppL.tile([128, 2], F32, tag="t")
                    nc.tensor.matmul(
                        dots_ps[:, 0:1], lq1b_w, lv_sb[:, 1:2],
                        start=True, stop=True,
                    )
                    nc.tensor.matmul(
                        dots_ps[:, 1:2], lq2b_w, lv_sb[:, 3:4],
                        start=True, stop=True,
                    )
                    eexp = consts.tile([128, 2], F32, tag="eexp")
                    nc.scalar.activation(eexp, dots_ps, EXP)
                    nc.vector.tensor_sub(lamc, eexp[:, 0:1], eexp[:, 1:2])
                    nc.vector.tensor_scalar_add(lamc, lamc, LAMBDA_INIT)
                    nc.sync.dma_start(padbc, padbc128.ap())

                # ============ phase 2 ============
                ATILE = 1024  # psA tile width (2 PSUM banks)
                with (
                    tc.tile_pool(name="psA", bufs=2, space="PSUM") as psA,
                    tc.tile_pool(name="sb2", bufs=2) as sb2,
                    tc.tile_pool(name="obufs", bufs=4) as obufs,
                    tc.tile_pool(name="rbufs", bufs=4) as rbufs,
                ):
                    def stage_a(j, premask=False):
                        """Scores -> mask -> exp into u bands, both branches.

                        premask: mask scores on PSUM before exp (1 DVE op) —
                        used for late k-tiles where ACT has slack; otherwise
                        mask u after exp (2 ops, off the score->exp path).
                        """
                        q_lo = 0 if j == 0 else 128 * (j - 1)
                        w = widths[j]
                        for br in range(2):
                            KT = (k1t, k2t)[br]
                            QT = (q1t, q2t)[br]
                            for t0 in range(0, w, ATILE):
                                tw = min(ATILE, w - t0)
                                s_ps = psA.tile([128, ATILE], F32, tag="s")
                                for n0 in range(0, tw, 512):
                                    nw = min(512, tw - n0)
                                    nc.tensor.matmul(
                                        s_ps[:, n0 : n0 + nw],
                                        KT[:, j * 128 : (j + 1) * 128],
                                        QT[:, q_lo + t0 + n0 : q_lo + t0 + n0 + nw],
                                        start=True,
                                        stop=True,
                                    )
                                mw = 128 if j == 0 else 256
                                moff = 128 if j == 0 else 0
                                if t0 == 0 and premask:
                                    nc.vector.tensor_mul(
                                        s_ps[:, 0:mw],
                                        s_ps[:, 0:mw],
                                        trisd_sb[:, moff : moff + mw],
                                    )
                                nc.scalar.activation(
                                    u_band[br][
                                        :, band_off[j] + t0 : band_off[j] + t0 + tw
                                    ],
                                    s_ps[:, :tw],
                                    EXP,
                                    scale=SCALE,
                                )
                                if t0 == 0 and not premask:
                                    # triangular mask applied AFTER exp, off
                                    # the score->exp critical path: u*m+(1-m)
                                    # forces masked positions to exactly 1
                                    useg = u_band[br][
                                        :, band_off[j] : band_off[j] + mw
                                    ]
                                    mask_eng.tensor_mul(
                                        useg, useg, trisdb[:, moff : moff + mw]
                                    )
                                    mask_eng.tensor_add(
                                        useg, useg, trisd1m[:, moff : moff + mw]
                                    )

                    # ---- V projection interleaved with stage A j=0..4 ----
                    with tc.tile_pool(name="ppV", bufs=1, space="PSUM") as ppV:
                        vps = ppV.tile([128, T], F32, tag="vproj")
                        vtb = obufs.tile([128, T], BF16, tag="vtb", bufs=1)
                        for ct in range(nC):
                            xt = xs.tile([128, T], BF16, tag="xt")
                            nc.sync.dma_start(xt, vT.ap()[ct * 128 : (ct + 1) * 128, :])
                            for n0 in range(0, T, 512):
                                nc.tensor.matmul(
                                    vps[:, n0 : n0 + 512],
                                    wv_w(ct),
                                    xt[:, n0 : n0 + 512],
                                    start=(ct == 0),
                                    stop=(ct == nC - 1),
                                )
                            if ct < 5 and not phase1_only:
                                stage_a(ct)
                        for n0 in range(0, T, 1024):
                            nc.vector.tensor_copy(
                                vtb[:, n0 : n0 + 1024], vps[:, n0 : n0 + 1024]
                            )

                    # V natural blocks + per-block column sums + suffix sums
                    with tc.tile_pool(name="ppT", bufs=2, space="PSUM") as ppT:
                        for j in range(nT):
                            vtr = ppT.tile([128, 128], BF16, tag="m")
                            nc.tensor.transpose(
                                vtr, vtb[:, j * 128 : (j + 1) * 128], idb_w
                            )
                            nc.vector.tensor_copy(vsb[:, j, :], vtr)
                    nc.vector.tensor_reduce(
                        vcols,
                        vtb.rearrange("p (j q) -> p j q", j=nT),
                        mybir.AxisListType.X,
                        mybir.AluOpType.add,
                    )
                    nc.vector.memset(sufv[:, nT - 1 : nT], 0.0)
                    nc.vector.memset(sufv[:, nT - 2 : nT - 1], 0.0)
                    for i in range(nT - 3, -1, -1):
                        nc.vector.tensor_add(
                            sufv[:, i : i + 1], sufv[:, i + 1 : i + 2],
                            vcols[:, i + 2 : i + 3],
                        )
                    with tc.tile_pool(name="ppS", bufs=1, space="PSUM") as ppS:
                        svt_ps = ppS.tile([16, 128], BF16, tag="svt")
                        nc.tensor.transpose(svt_ps, sufv.bitcast(F32), idb_w)
                        nc.vector.tensor_copy(sufvT, svt_ps)
                    nc.vector.tensor_add(tvn, sufv[:, 0:1], vcols[:, 0:1])
                    nc.vector.tensor_add(tvn, tvn, vcols[:, 1:2])
                    nc.vector.tensor_scalar_mul(tvn, tvn, 1.0 / T)
                    # pad rows see uniform attention in both branches:
                    # out = (1 - lambda) * mean(V)
                    tvl = consts.tile([128, 1], F32, tag="tvl")
                    nc.vector.tensor_scalar(
                        tvl, tvn, lamc, None, mybir.AluOpType.mult
                    )
                    nc.vector.tensor_sub(tvn, tvn, tvl)

                    # ---- stage B + remaining stage A groups ----
                    pending = []

                    def emit_combine(q0, br_res, last=False):
                        # oT = O1*R1 - lambda*O2*R2; pad rows -> uniform value.
                        # The final chunk's combine skips the Pool handoff: PE
                        # is done, so the shortest serial chain wins.
                        ceng = nc.vector if last else comb_eng
                        (ou1sb, r1bc), (ou2sb, r2bc) = br_res
                        c1 = sb2.tile([128, NQ], F32, tag="c1")
                        ceng.tensor_mul(c1, ou1sb, r1bc)
                        c2 = sb2.tile([128, NQ], F32, tag="c2")
                        nc.vector.scalar_tensor_tensor(
                            c2, ou2sb, lamc, r2bc,
                            mybir.AluOpType.mult, mybir.AluOpType.mult,
                        )
                        oT = sb2.tile([128, NQ], F32, tag="oT")
                        ceng.tensor_sub(oT, c1, c2)
                        oT2 = sb2.tile([128, NQ], F32, tag="oT2")
                        nc.vector.scalar_tensor_tensor(
                            oT2, padbc[:, q0 : q0 + NQ], tvn, oT,
                            mybir.AluOpType.mult, mybir.AluOpType.add,
                        )
                        nc.sync.dma_start(out.ap()[:, q0 : q0 + NQ], oT2)

                    def b_window(c, j):
                        q0 = c * NQ
                        q_lo = 0 if j == 0 else 128 * (j - 1)
                        oo = max(0, q_lo - q0)
                        wo = max(0, q0 - q_lo)
                        return q0, oo, wo, NQ - oo

                    class BChunk:
                        """One query chunk of stage B, emitted in j-slices so
                        the PE stream can alternate with stage-A groups."""

                        def __init__(self, c, psO, psD):
                            self.c = c
                            self.psD = psD
                            self.q0 = c * NQ
                            self.jmax = min((self.q0 + NQ) // 128, nT - 1)
                            if pending:
                                emit_combine(*pending.pop(0))
                            self.ou_h = [
                                psO.tile([128, NQ], F32, tag="ou", name=f"ou{br}")
                                for br in range(2)
                            ]
                            self.d2ps = psD.tile([128, NQ], F32, tag="dd")
                            # masked-count row seeds the denominator chain
                            for br in range(2):
                                nc.tensor.matmul(
                                    self.d2ps[32 * br : 32 * br + 1, :],
                                    onesb_w[0:1, :],
                                    sufb_sb[:, self.q0 : self.q0 + NQ],
                                    start=True,
                                    stop=False,
                                    skip_group_check=True,
                                )

                        def dsum_slice(self, js):
                            for br in range(2):
                                dps = self.d2ps[32 * br : 32 * br + 1, :]
                                for j in js:
                                    _, oo, wo, w = b_window(self.c, j)
                                    nc.tensor.matmul(
                                        dps[:, oo : oo + w],
                                        onesb_w,
                                        u_band[br][
                                            :, band_off[j] + wo : band_off[j] + wo + w
                                        ],
                                        start=False,
                                        stop=(j == self.jmax),
                                        skip_group_check=True,
                                    )

                        def uv_slice(self, js):
                            final = self.c == T // NQ - 1
                            for j in js:
                                _, oo, wo, w = b_window(self.c, j)
                                for br in range(2):
                                    nc.tensor.matmul(
                                        self.ou_h[br][:, oo : oo + w],
                                        vsb[:, j, :],
                                        u_band[br][
                                            :, band_off[j] + wo : band_off[j] + wo + w
                                        ],
                                        start=(j == 0),
                                        stop=(j == self.jmax and not final),
                                        skip_group_check=True,
                                    )
                            if final and js and js[-1] == self.jmax:
                                # fold the suffix-V add in on the PE so the
                                # final combine can read O_u straight from
                                # PSUM with no DVE drain
                                i0 = self.q0 // 128
                                for br in range(2):
                                    nc.tensor.matmul(
                                        self.ou_h[br],
                                        sufvT[i0 : i0 + 4, :],
                                        blkind_w,
                                        start=False,
                                        stop=True,
                                        skip_group_check=True,
                                    )

                        def slice(self, js, last=False):
                            # denominators first: on the last slice their
                            # reciprocal/broadcast chain then overlaps the
                            # remaining U@V matmuls
                            self.dsum_slice(js)
                            if last:
                                self.epi_d()
                            self.uv_slice(js)

                        def epi_d(self):
                            q0 = self.q0
                            self.rbcs = []
                            for br in range(2):
                                dps = self.d2ps[32 * br : 32 * br + 1, :]
                                rrow = sb2.tile([1, NQ], BF16, tag="rrow")
                                with nc.allow_low_precision(
                                    reason="bf16 recip row feeds a bf16 "
                                    "broadcast matmul; 0.4% is within budget"
                                ):
                                    nc.vector.reciprocal(rrow, dps)
                                    # zero padded cols (pad handled in combine)
                                    nc.vector.tensor_mul(
                                        rrow, rrow, npad_sb[:, q0 : q0 + NQ]
                                    )
                                rb_ps = self.psD.tile([128, NQ], F32, tag="rb")
                                nc.tensor.matmul(
                                    rb_ps, onesr_bw, rrow, start=True, stop=True
                                )
                                rbc = rbufs.tile([128, NQ], F32, tag="rbc")
                                if br == 0:
                                    nc.vector.tensor_copy(rbc, rb_ps)
                                else:
                                    nc.scalar.copy(rbc, rb_ps)
                                self.rbcs.append(rbc)

                        def epilogue(self):
                            q0 = self.q0
                            if self.c == T // NQ - 1:
                                # final chunk: combine reads O_u directly from
                                # PSUM (suffix sums were folded in on the PE)
                                pending.append(
                                    (q0, [(self.ou_h[b], self.rbcs[b]) for b in range(2)])
                                )
                                return
                            # drain O_u (adding skipped-region suffix V sums),
                            # defer the combine
                            br_res = []
                            for br in range(2):
                                ousb = obufs.tile([128, NQ], F32, tag="ousb")
                                nb = NQ // 128
                                nc.vector.tensor_tensor(
                                    ousb.rearrange("p (b q) -> p b q", b=nb),
                                    self.ou_h[br].rearrange("p (b q) -> p b q", b=nb),
                                    sufv[:, q0 // 128 : q0 // 128 + nb]
                                    .unsqueeze(2)
                                    .broadcast_to([128, nb, 128]),
                                    mybir.AluOpType.add,
                                )
                                br_res.append((ousb, self.rbcs[br]))
                            pending.append((q0, br_res))

                    if not phase1_only:
                        with (
                            tc.tile_pool(name="psO", bufs=2, space="PSUM") as psO,
                            tc.tile_pool(name="psD", bufs=1, space="PSUM") as psD,
                        ):
                            # Fine-grained interleave: single A groups alternate
                            # with j-slices of the one active B chunk, so the
                            # in-order PE stream never sits long behind the ACT
                            # exp queue (psA is only 2 tiles deep).
                            B = lambda c: BChunk(c, psO, psD)
                            b = B(0)
                            stage_a(5)
                            stage_a(6)
                            b.slice(range(0, 3))
                            stage_a(7)
                            b.slice(range(3, 5), last=True)
                            b.epilogue()
                            stage_a(8)
                            b = B(1)
                            b.slice(range(0, 3))
                            stage_a(9)
                            b.slice(range(3, 6))
                            stage_a(10)
                            b.slice(range(6, 9), last=True)
                            b.epilogue()
                            stage_a(11)
                            b = B(2)
                            b.slice(range(0, 4))
                            stage_a(12)
                            b.slice(range(4, 8))
                            stage_a(13)
                            b.slice(range(8, 13), last=True)
                            b.epilogue()
                            stage_a(14, premask=True)
                            stage_a(15, premask=True)
                            b = B(3)
                            b.slice(range(0, 13))
                            b.slice(range(13, nT), last=True)
                            b.epilogue()
                            for args in pending:
                                emit_combine(*args, last=True)
                            pending = []

    nc.compile()
    return nc


def _host_constants(T, nT):
    kl = np.arange(128)[:, None]
    ql = np.arange(128)[None, :]
    trid = (kl <= ql + 1).astype(np.float32)
    tris = ((kl + 128) <= (ql + 1)).astype(np.float32)
    tiles = np.arange(T) // 128
    sufcnt = (128.0 * np.maximum(0, nT - 2 - tiles)).astype(np.float32)[None, :]
    return trid, tris, sufcnt


_NC_CACHE = {}


def make_in_maps(q, k, v, pad_mask, Wq, Wk, Wv, lq1, lk1, lq2, lk2):
    """Per-core input dicts (host-side sharding + layout marshaling)."""
    B, T, C = q.shape
    nT, nC = T // 128, C // 128
    bf16 = ml_dtypes.bfloat16
    trid, tris, sufcnt = _host_constants(T, nT)

    def blocked(W):
        # [C, D] -> [128, nC*D] with cols ct*D + d = W[ct*128 + p, d]
        D = W.shape[1]
        return (
            np.asarray(W)
            .reshape(nC, 128, D)
            .transpose(1, 0, 2)
            .reshape(128, nC * D)
        )

    # block indicator: rows 0..3 mark which 128-col block each column is in
    blkind = np.zeros((128, 512), dtype=np.float32)
    for p in range(4):
        blkind[p, p * 128 : (p + 1) * 128] = 1.0
    cbK = np.ascontiguousarray(blocked(Wk)).astype(bf16)
    cbR = np.concatenate(
        [
            blocked(Wq),
            blocked(Wv),
            np.eye(128, dtype=np.float32),
            np.ones((128, 1), dtype=np.float32),
            np.ones((128, 128), dtype=np.float32),
            np.concatenate([tris, trid], axis=1),
            1.0 - np.concatenate([tris, trid], axis=1),
            blkind,
        ],
        axis=1,
    ).astype(bf16)
    lvec = np.stack(
        [np.asarray(lq1), np.asarray(lk1), np.asarray(lq2), np.asarray(lk2)], axis=1
    ).astype(np.float32)
    lq1b = np.tile(np.asarray(lq1)[:, None], (1, 128))
    lq2b = np.tile(np.asarray(lq2)[:, None], (1, 128))
    cbf32 = np.concatenate([tris, trid, lvec, lq1b, lq2b], axis=1).astype(
        np.float32
    )
    sufcnt_bf = np.ascontiguousarray(sufcnt.astype(bf16))

    in_maps = []
    for b in range(B):
        padf = np.asarray(pad_mask[b], dtype=np.float32)
        pn2 = np.ascontiguousarray(
            np.concatenate([padf, 1.0 - padf])[None, :].astype(bf16)
        )
        padbc128 = np.ascontiguousarray(
            np.broadcast_to(padf[None, :], (128, T)).astype(bf16)
        )
        in_maps.append(
            dict(
                padbc128=padbc128,
                qT=np.ascontiguousarray(np.asarray(q[b]).T.astype(bf16)),
                kT=np.ascontiguousarray(np.asarray(k[b]).T.astype(bf16)),
                vT=np.ascontiguousarray(np.asarray(v[b]).T.astype(bf16)),
                cbK=cbK,
                cbR=cbR,
                cbf32=cbf32,
                pn2=pn2,
                sufcnt_bf=sufcnt_bf,
            )
        )
    return in_maps


def kernel(q, k, v, pad_mask, Wq, Wk, Wv, lq1, lk1, lq2, lk2):
    B, T, C = q.shape
    assert B == N_CORES
    key = (T, C)
    if key not in _NC_CACHE:
        _NC_CACHE[key] = build_nc(T=T, C=C)
    nc = _NC_CACHE[key]
    in_maps = make_in_maps(q, k, v, pad_mask, Wq, Wk, Wv, lq1, lk1, lq2, lk2)
    res = run_bass_kernel_spmd(nc, in_maps, core_ids=list(range(N_CORES)))
    return np.stack(
        [np.ascontiguousarray(r["out"].T) for r in res.results], axis=0
    )
